# revision 19
# baseline (speedup 1.0000x reference)
"""Bipartite GNN encoder (SAGEConv x2 layers) as a Trainium2 Bass/Tile SPMD kernel.

v2 strategy (8 cores):
  - Destination-sharded message passing, linear layers folded into per-node
    transforms (y = x @ ll_w gathered as messages; accum init z = x@lr_w + b).
  - bf16 conv path: y tables stored [N, 128] bf16 (row-padded to 256B so
    dma_gather's 256B-elem constraint is met); messages scaled in bf16;
    dma_scatter_add in bf16 (128B descriptors -> half DMA time) into bf16
    SBUF accumulators.
  - cons embedding is REPLICATED on every core (input table is small) with a
    per-core ROTATED section layout (SPMD program identical; per-core data
    differs) so z_c0 always comes from rotated-section 0.  This removes the
    y_c0 AllGather entirely.
  - var embedding stays sharded + AllGather (y_v0 AG hides under cv0 DMA).
  - Wide (512-row) embed pipeline using host-transposed inputs: no PE
    transposes, ~4x fewer instructions than per-128-tile embeds.
  - Same-dst tokens never share one scatter call (HW RMW race): streams
    sorted by (src_chunk, dst) and strided across >= max_run slots.
  - Layer-1 v->c conv is skipped (its output is unused by the reference).
"""
import sys
sys.path.insert(0, "/opt/trn_rl_repo")
import numpy as np
import concourse.bass as bass
import concourse.bacc as bacc
import concourse.mybir as mybir
import concourse.tile as tile
from concourse.masks import make_identity

F32 = mybir.dt.float32
BF16 = mybir.dt.bfloat16
I16 = mybir.dt.int16
NPBF = mybir.dt.np(mybir.dt.bfloat16)
P = 128
EMB = 64
YW = 128          # padded y-table row width (256B in bf16)
RWIDE = 1024      # wide-embed block rows


def pad_to(n, m):
    return (n + m - 1) // m * m


# ---------------------------------------------------------------- host prep

def pack_idx16(a, cap, pad_val):
    b = np.full(cap, pad_val, np.int64)
    b[: len(a)] = a
    assert b.max() < 32768 and b.min() >= 0
    m = b.astype(np.int16).reshape(cap // 16, 16).T  # token j -> [j%16, j//16]
    return np.tile(m, (8, 1))  # replicate for the 8 q7 cores


def pack_vals(a, cap, dtype):
    b = np.zeros(cap, np.float32)
    b[: len(a)] = a
    return b.reshape(cap // 128, 128).T.astype(dtype).copy()


class ConvPlan:
    """Token stream plan for one conv direction, shared static structure across cores.

    src_gp_list: per-core arrays of global-padded source rows (per-core table
    layouts may differ, e.g. rotated cons tables)."""

    def __init__(self, src_gp_list, dst_g, n_dst, dst_sh_real, dst_sh_pad, src_rows_pad,
                 ncores, chunk, cap_target, edge_mask=None, dst_loc_arr=None,
                 trash=None, chunk_bounds=None):
        self.dst_sh_pad = dst_sh_pad
        owner = dst_g // dst_sh_real
        dst_loc = dst_loc_arr if dst_loc_arr is not None else dst_g - owner * dst_sh_real
        if chunk_bounds is None:
            n_chunks = pad_to(src_rows_pad, chunk) // chunk
            chunk_bounds = [c * chunk for c in range(n_chunks)] + [src_rows_pad]
        n_chunks = len(chunk_bounds) - 1
        cb = np.asarray(chunk_bounds)
        per_core = []  # per core: list over chunks of (gidx_sorted, dloc_sorted, inv_sorted)
        cnt = np.zeros((ncores, n_chunks), np.int64)
        runmax = np.zeros(n_chunks, np.int64)
        for k in range(ncores):
            m = owner == k
            if edge_mask is not None:
                m = m & edge_mask
            src_gp = src_gp_list[k]
            gp_k, dl_k = src_gp[m], dst_loc[m]
            c_k = np.searchsorted(cb, gp_k, side="right") - 1
            order = np.lexsort((dl_k, c_k))
            gp_k, dl_k, c_k = gp_k[order], dl_k[order], c_k[order]
            rows = []
            for c in range(n_chunks):
                mc = c_k == c
                g, d = gp_k[mc] - cb[c], dl_k[mc]
                cnt[k, c] = len(g)
                if len(d):
                    # longest run of equal dst
                    brk = np.flatnonzero(np.diff(d) != 0)
                    edges = np.concatenate([[-1], brk, [len(d) - 1]])
                    runmax[c] = max(runmax[c], np.diff(edges).max())
                rows.append((g, d))
            per_core.append(rows)
        # static slot structure
        self.slots = []  # list of (chunk, cap, src_row_base, src_rows_in_chunk)
        self.nslots_c = []
        for c in range(n_chunks):
            cmax = cnt[:, c].max()
            if cmax == 0:
                self.nslots_c.append(0)
                continue
            ns = int(max(-(-cmax // cap_target), runmax[c], 1))
            cap = pad_to(-(-cmax // ns), 128)
            rows_c = int(cb[c + 1] - cb[c])
            for j in range(ns):
                self.slots.append((c, int(cap), int(cb[c]), rows_c))
            self.nslots_c.append(ns)
        self.tot16 = sum(cap for _, cap, _, _ in self.slots) // 16
        self.tot128 = sum(cap for _, cap, _, _ in self.slots) // 128
        # per-core packed streams
        self.gidx, self.sidx = [], []
        if trash is None:
            trash = dst_sh_pad - 1
        for k in range(ncores):
            gs, ss = [], []
            for c in range(n_chunks):
                ns = self.nslots_c[c]
                if ns == 0:
                    continue
                g, d = per_core[k][c]
                cap = [cp for (cc, cp, _, _) in self.slots if cc == c][0]
                for j in range(ns):
                    gj, dj = g[j::ns], d[j::ns]
                    assert len(gj) <= cap
                    gs.append(pack_idx16(gj, cap, 0))
                    ss.append(pack_idx16(dj, cap, trash))
            self.gidx.append(np.concatenate(gs, axis=1))
            self.sidx.append(np.concatenate(ss, axis=1))


class Problem:
    def __init__(self, n_cons, n_var, cons_nf, var_nf, ncores=8, chunk=32768,
                 cap_target=9216, nrep=2):
        self.ncores, self.chunk, self.cap_target, self.nrep = ncores, chunk, cap_target, nrep
        self.n_cons, self.n_var, self.cons_nf, self.var_nf = n_cons, n_var, cons_nf, var_nf
        assert n_cons % ncores == 0 and n_var % ncores == 0
        self.Csh = n_cons // ncores
        self.Vsh = n_var // ncores
        self.CshP = pad_to(self.Csh + 1, 2 * P)  # +1: last row is scatter-pad trash
        self.VshP = pad_to(self.Vsh + 1, 2 * P)
        self.Cfull = self.CshP * ncores
        self.Vfull = self.VshP * ncores
        self.VshPP = pad_to(self.VshP, 1024)   # wide-embed block padding
        self.CshPP = pad_to(self.CshP, 1024)
        self.VhR = self.VshPP // 2             # v-shard region half (12800)
        self.VfullR = self.VshPP * ncores      # v-table region-layout rows

    # cons-section local rows are "half-mapped": halves of each section are
    # padded independently so the v->c conv + its AllGather can be split into
    # two pipelined halves.  device-local row r = halfmap(l).
    def halfmap(self, l):
        hA, hAp = self.Csh // 2, self.CshP // 2
        return np.where(l < hA, l, hAp + (l - hA))

    def gp_cons_rot(self, idx, k):
        # per-core rotated cons layout: global section s sits at position (s-k)%ncores
        sec = idx // self.Csh
        return ((sec - k) % self.ncores) * self.CshP + self.halfmap(idx % self.Csh)

    def gp_cons_region(self, idx):
        # y_c1 layout: [all A-halves (8 x CshP/2 rows) | all B-halves]
        hA, hAp = self.Csh // 2, self.CshP // 2
        sec, l = idx // self.Csh, idx % self.Csh
        return np.where(l < hA, sec * hAp + l,
                        self.ncores * hAp + sec * hAp + (l - hA))

    def gp_var(self, idx):
        # y_v0 layout: [all A-halves (8 x VhR rows) | all B-halves]
        k, l = idx // self.Vsh, idx % self.Vsh
        return np.where(l < self.VhR, k * self.VhR + l,
                        self.ncores * self.VhR + k * self.VhR + (l - self.VhR))

    def prep(self, edge_index):
        src, dst = np.asarray(edge_index[0]), np.asarray(edge_index[1])
        nc_ = self.ncores
        hA, hAp = self.Csh // 2, self.CshP // 2
        self.cv0 = ConvPlan([self.gp_cons_rot(src, k) for k in range(nc_)],
                            dst, self.n_var, self.Vsh, self.VshP,
                            self.Cfull, nc_, self.chunk, self.cap_target)
        gpc = self.gp_cons_region(src)
        half_rows = nc_ * hAp
        cb = [0, 32768, half_rows, half_rows + 32768, 2 * half_rows]
        self.cv1 = ConvPlan([gpc] * nc_, dst, self.n_var, self.Vsh, self.VshP,
                            self.Cfull, nc_, self.chunk, self.cap_target,
                            chunk_bounds=cb)
        self.cv1_a_slots = sum(self.cv1.nslots_c[:2])
        gpv = self.gp_var(dst)
        cl = src % self.Csh
        hm = self.halfmap(cl)
        vhalf = self.ncores * self.VhR
        vcb = ([0, 32768, 65536, 98304, vhalf, vhalf + 32768, vhalf + 65536,
                vhalf + 98304, 2 * vhalf])
        self.vcA = ConvPlan([gpv] * nc_, src, self.n_cons, self.Csh, self.CshP,
                            self.VfullR, nc_, self.chunk, self.cap_target,
                            edge_mask=cl < hA, dst_loc_arr=hm, trash=hAp - 1,
                            chunk_bounds=vcb)
        self.vcB = ConvPlan([gpv] * nc_, src, self.n_cons, self.Csh, self.CshP,
                            self.VfullR, nc_, self.chunk, self.cap_target,
                            edge_mask=cl >= hA, dst_loc_arr=hm - hAp, trash=hAp - 1,
                            chunk_bounds=vcb)

    # ------------------------------------------------------------ in_maps
    def in_maps(self, inputs):
        ii = {k: np.asarray(v) for k, v in inputs.items()}
        maps = []
        rep4 = lambda b: np.tile(np.repeat(np.asarray(b, np.float32)[None, :], P, 0),
                                 (1, 4))  # [128, 4*64] bias tile
        cons_x = ii["cons_x"]
        var_x = ii["var_x"]
        for k in range(self.ncores):
            cxf = np.zeros((self.Cfull, self.cons_nf), np.float32)
            hm = self.halfmap(np.arange(self.Csh))
            for s in range(self.ncores):
                rot = (s - k) % self.ncores
                cxf[rot * self.CshP + hm] = cons_x[s * self.Csh:(s + 1) * self.Csh]
            vx = np.zeros((self.VshPP, self.var_nf), np.float32)
            vx[: self.Vsh] = var_x[k * self.Vsh:(k + 1) * self.Vsh]
            # per-dst-row 1/deg tables, laid out like the accumulators
            def inv_tiles(deg_full, sh_real, sh_pad, loc_map=None):
                iv = np.ones(sh_pad, np.float32)
                loc = loc_map if loc_map is not None else np.arange(sh_real)
                iv[loc] = 1.0 / np.maximum(deg_full[k * sh_real:(k + 1) * sh_real], 1)
                t = iv.reshape(-1, P).T  # [P, ntiles]
                return (np.ascontiguousarray(t[:, 0::2]).astype(NPBF),
                        np.ascontiguousarray(t[:, 1::2]).astype(NPBF))
            deg_v = np.bincount(np.asarray(ii["edge_index"][1]), minlength=self.n_var)
            deg_c = np.bincount(np.asarray(ii["edge_index"][0]), minlength=self.n_cons)
            cvi_e, cvi_o = inv_tiles(deg_v, self.Vsh, self.VshP)
            vci_e, vci_o = inv_tiles(deg_c, self.Csh, self.CshP,
                                     loc_map=self.halfmap(np.arange(self.Csh)))
            m = {
                "cons_xT": np.ascontiguousarray(cxf.T).astype(NPBF),
                "var_xT": np.ascontiguousarray(vx.T).astype(NPBF),
                # PreNorm folded into the first linear: (x+s)*sc @ w1 + b1
                # == x @ (diag(sc) w1) + (b1 + (s*sc) @ w1)
                "cons_w1": (ii["cons_scale"][:, None] * ii["cons_w1"]).astype(NPBF),
                "cons_b1": (ii["cons_b1"] + (ii["cons_shift"] * ii["cons_scale"])
                            @ ii["cons_w1"]).reshape(-1, 1),
                "cons_w2": ii["cons_w2"].astype(NPBF), "cons_b2": ii["cons_b2"].reshape(-1, 1),
                "var_w1": (ii["var_scale"][:, None] * ii["var_w1"]).astype(NPBF),
                "var_b1": (ii["var_b1"] + (ii["var_shift"] * ii["var_scale"])
                           @ ii["var_w1"]).reshape(-1, 1),
                "var_w2": ii["var_w2"].astype(NPBF), "var_b2": ii["var_b2"].reshape(-1, 1),
                "ll_w00": ii["conv_ll_w"][0, 0].astype(NPBF), "lr_w00": ii["conv_lr_w"][0, 0].astype(NPBF),
                "ll_w01": ii["conv_ll_w"][0, 1].astype(NPBF), "lr_w01": ii["conv_lr_w"][0, 1].astype(NPBF),
                "ll_w10b": ii["conv_ll_w"][1, 0].astype(NPBF),
                "lr_w10b": ii["conv_lr_w"][1, 0].astype(NPBF),
                "btz_c": rep4(ii["conv_ll_b"][0, 1]),   # bias for z_c0 (vc0 init)
                "btz_v": rep4(ii["conv_ll_b"][0, 0]),   # bias for z_v0 (cv0 init)
                "bt10_4": rep4(ii["conv_ll_b"][1, 0]),  # bias for z_v1 (cv1 init)
                "cvi_e": cvi_e, "cvi_o": cvi_o, "vci_e": vci_e, "vci_o": vci_o,
                "c0_gidx": self.cv0.gidx[k], "c0_sidx": self.cv0.sidx[k],
                "c1_gidx": self.cv1.gidx[k], "c1_sidx": self.cv1.sidx[k],
                "va_gidx": self.vcA.gidx[k], "va_sidx": self.vcA.sidx[k],
                "vb_gidx": self.vcB.gidx[k], "vb_sidx": self.vcB.sidx[k],
            }
            maps.append(m)
        return maps

    # ------------------------------------------------------------ kernel
    def build(self, dbg=False):
        pr = self
        nc = bacc.Bacc("TRN2", target_bir_lowering=False, debug=False)
        dp = lambda n, s, d=F32: nc.declare_dram_parameter(n, s, d, isOutput=False)
        cons_xT = dp("cons_xT", [pr.cons_nf, pr.Cfull], BF16)
        var_xT = dp("var_xT", [pr.var_nf, pr.VshPP], BF16)
        w = {}
        for nm, s, d in [("cons_w1", [pr.cons_nf, EMB], BF16), ("cons_b1", [EMB, 1], F32),
                         ("cons_w2", [EMB, EMB], BF16), ("cons_b2", [EMB, 1], F32),
                         ("var_w1", [pr.var_nf, EMB], BF16), ("var_b1", [EMB, 1], F32),
                         ("var_w2", [EMB, EMB], BF16), ("var_b2", [EMB, 1], F32),
                         ("ll_w00", [EMB, EMB], BF16), ("lr_w00", [EMB, EMB], BF16),
                         ("ll_w01", [EMB, EMB], BF16), ("lr_w01", [EMB, EMB], BF16),
                         ("ll_w10b", [EMB, EMB], BF16), ("lr_w10b", [EMB, EMB], BF16),
                         ("btz_c", [P, 8 * EMB], F32), ("btz_v", [P, 8 * EMB], F32),
                         ("bt10_4", [P, 8 * EMB], F32)]:
            w[nm] = dp(nm, s, d)
        streams = {}
        for pre, plan in [("c0", pr.cv0), ("c1", pr.cv1), ("va", pr.vcA), ("vb", pr.vcB)]:
            streams[pre + "_gidx"] = dp(pre + "_gidx", [P, plan.tot16], I16)
            streams[pre + "_sidx"] = dp(pre + "_sidx", [P, plan.tot16], I16)
        for nm, n in [("cvi_e", pr.VshP // 256), ("cvi_o", pr.VshP // 256),
                      ("vci_e", pr.CshP // 256), ("vci_o", pr.CshP // 256)]:
            w[nm] = dp(nm, [P, n], BF16)
        out_xv2 = nc.declare_dram_parameter("out_xv2", [pr.VshP, EMB], F32, isOutput=True)

        y_c0 = nc.dram_tensor("y_c0", [pr.Cfull, YW], BF16)
        y_v0_sh = nc.dram_tensor("y_v0_sh", [pr.VshPP, YW], BF16)
        y_c1_sh = nc.dram_tensor("y_c1_sh", [pr.CshP, YW], BF16)
        y_v0 = nc.dram_tensor("y_v0", [pr.VfullR, YW], BF16, addr_space="Shared")
        y_c1 = nc.dram_tensor("y_c1", [pr.Cfull, YW], BF16, addr_space="Shared")
        z_v0 = nc.dram_tensor("z_v0", [pr.VshPP, EMB], BF16)
        z_c0 = nc.dram_tensor("z_c0", [pr.CshPP, EMB], BF16)
        z_v1 = nc.dram_tensor("z_v1", [pr.VshPP, EMB], BF16)

        RG = [list(range(pr.ncores))]
        NREP = pr.nrep

        with tile.TileContext(nc) as tc:
            with tc.tile_pool(name="const", bufs=1) as constp, \
                 tc.tile_pool(name="work", bufs=3) as workp, \
                 tc.tile_pool(name="msg", bufs=2) as msgp, \
                 tc.tile_pool(name="accp", bufs=1) as accp, \
                 tc.tile_pool(name="psum_e", bufs=1, space="PSUM") as psum_e, \
                 tc.tile_pool(name="psum_t", bufs=1, space="PSUM") as psum_t:

                def ag_act(in_ap, out_ap):
                    bass.BassGpSimd.collective_compute(
                        nc.scalar, "AllGather", mybir.AluOpType.bypass,
                        ins=[in_ap], outs=[out_ap], replica_groups=RG)

                ident = constp.tile([P, P], BF16)
                make_identity(nc, ident[:])
                wsb = {}
                for nm in w:
                    t = constp.tile(list(w[nm].shape), w[nm].dtype, tag=f"w_{nm}",
                                    name=f"wsb_{nm}")
                    nc.sync.dma_start(out=t[:], in_=w[nm][:])
                    wsb[nm] = t

                def embed_wide(xT_dram, nf, nrows, w1, b1, w2, b2,
                               yw, y_dst, zw, zbias4, z_dst, z_rows,
                               blocks=None, use_act=True):
                    """Wide embed: per 1024-row block, MLP + y/z transforms.

                    xT_dram: [nf, nrows] host-transposed input. y_dst: [nrows, YW]
                    bf16 (cols 0:EMB written). z written for rows < z_rows.
                    use_act=False runs the two relus on DVE (bias-add + max) so
                    the Activation engine stays free for collectives."""
                    G = RWIDE // P
                    nblk = -(-nrows // RWIDE)
                    yv = y_dst.ap()[:, 0:EMB].rearrange("(b i p) e -> b p i e", p=P, i=G)
                    zv = z_dst.ap().rearrange("(b i p) e -> b p i e", p=P, i=G)
                    for bi in (blocks if blocks is not None else range(nblk)):
                        rb = bi * RWIDE
                        xin = workp.tile([nf, RWIDE], BF16, tag="e_xin")
                        nc.sync.dma_start(out=xin[:], in_=xT_dram[:, rb:rb + RWIDE])
                        h1 = psum_e.tile([EMB, RWIDE], F32, tag="e_h1")
                        for s in range(RWIDE // 512):
                            nc.tensor.matmul(out=h1[:, s * 512:(s + 1) * 512],
                                             lhsT=wsb[w1][:],
                                             rhs=xin[:, s * 512:(s + 1) * 512],
                                             start=True, stop=True)
                        h1s = workp.tile([EMB, RWIDE], BF16, tag="e_h1s")
                        nc.vector.tensor_scalar(
                            out=h1s[:], in0=h1[:], scalar1=wsb[b1][:], scalar2=0.0,
                            op0=mybir.AluOpType.add, op1=mybir.AluOpType.max)
                        h2 = psum_e.tile([EMB, RWIDE], F32, tag="e_h2")
                        for s in range(RWIDE // 512):
                            nc.tensor.matmul(out=h2[:, s * 512:(s + 1) * 512],
                                             lhsT=wsb[w2][:],
                                             rhs=h1s[:, s * 512:(s + 1) * 512],
                                             start=True, stop=True)
                        xT = workp.tile([EMB, RWIDE], BF16, tag="e_xT")
                        nc.vector.tensor_scalar(
                            out=xT[:], in0=h2[:], scalar1=wsb[b2][:], scalar2=0.0,
                            op0=mybir.AluOpType.add, op1=mybir.AluOpType.max)
                        yp = psum_e.tile([P, G, EMB], F32, tag="e_yp")
                        for i in range(G):
                            nc.tensor.matmul(out=yp[:, i, :],
                                             lhsT=xT[:, i * P:(i + 1) * P],
                                             rhs=wsb[yw][:], start=True, stop=True)
                        ys = workp.tile([P, G, EMB], BF16, tag="e_ys")
                        nc.vector.tensor_copy(out=ys[:], in_=yp[:])
                        nc.sync.dma_start(out=yv[bi], in_=ys[:])
                        zg = min(G, max(0, (z_rows - rb) // P))
                        if zg > 0:
                            zp = psum_e.tile([P, G, EMB], F32, tag="e_zp")
                            for i in range(zg):
                                nc.tensor.matmul(out=zp[:, i, :],
                                                 lhsT=xT[:, i * P:(i + 1) * P],
                                                 rhs=wsb[zw][:], start=True, stop=True)
                            zs = workp.tile([P, G, EMB], BF16, tag="e_zs")
                            nc.vector.tensor_tensor(
                                out=zs[:, :zg, :], in0=zp[:, :zg, :],
                                in1=wsb[zbias4][:].rearrange("p (g e) -> p g e", e=EMB)[:, :zg, :],
                                op=mybir.AluOpType.add)
                            nc.sync.dma_start(out=zv[bi][:, :zg, :], in_=zs[:, :zg, :])

                def open_acc(ntiles, tagp, epad=0):
                    """Alloc e/o accumulators (+NREP replicas), zeroed.
                    z init is added at finalize (after the 1/deg multiply)."""
                    ge = -(-ntiles // 2)
                    go = ntiles // 2
                    acc_e = [accp.tile([P, ge + epad, EMB], BF16, tag=f"acc_e{r}",
                                       name=f"{tagp}_acc_e{r}") for r in range(NREP)]
                    acc_o = [accp.tile([P, go, EMB], BF16, tag=f"acc_o{r}",
                                       name=f"{tagp}_acc_o{r}") for r in range(NREP)]
                    for r in range(NREP):
                        nc.vector.memset(acc_e[r][:], 0.0)
                        nc.vector.memset(acc_o[r][:], 0.0)
                    return acc_e, acc_o

                def load_zrep(z_dram, ntiles):
                    zv = z_dram.ap().rearrange("(t p) d -> p t d", p=P)
                    ze = accp.tile([P, -(-ntiles // 2), EMB], BF16, tag="zrep_e")
                    zo = accp.tile([P, ntiles // 2, EMB], BF16, tag="zrep_o")
                    nc.sync.dma_start(out=ze[:], in_=zv[:, 0:ntiles:2, :])
                    nc.sync.dma_start(out=zo[:], in_=zv[:, 1:ntiles:2, :])
                    return ze, zo

                def emit_slots(plan, pre, own_fn, order=None):
                    """own_fn(r) -> (own_ap, other_ap) scatter targets.
                    Messages accumulate UNSCALED; 1/deg applied at finalize."""
                    gidx_d = streams[pre + "_gidx"]
                    sidx_d = streams[pre + "_sidx"]
                    offs, o16 = [], 0
                    for (c, cap, base, rows_c) in plan.slots:
                        offs.append(o16)
                        o16 += cap // 16
                    idxs = order if order is not None else range(len(plan.slots))
                    for si_, i in enumerate(idxs):
                        c, cap, base, rows_c = plan.slots[i]
                        f16 = offs[i]
                        gi = workp.tile([P, cap // 16], I16, tag="cv_gi")
                        sx = workp.tile([P, cap // 16], I16, tag="cv_si")
                        nc.sync.dma_start(out=gi[:], in_=gidx_d[:, f16:f16 + cap // 16])
                        nc.sync.dma_start(out=sx[:], in_=sidx_d[:, f16:f16 + cap // 16])
                        msgs = msgp.tile([P, cap // 128, YW], BF16, tag="cv_msg")
                        nc.gpsimd.dma_gather(
                            out_ap=msgs[:], in_ap=plan.y_full[base:base + rows_c, :],
                            idxs_ap=gi[:], num_idxs=cap, num_idxs_reg=cap, elem_size=YW,
                            single_packet=False)
                        sc = msgp.tile([P, cap // 128, EMB], BF16, tag="cv_sc")
                        nc.vector.tensor_copy(out=sc[:], in_=msgs[:, :, 0:EMB])
                        r = si_ % NREP
                        own, other = own_fn(r)
                        nc.gpsimd.dma_scatter_add(
                            out_ap=own, in_ap=sc[:], idxs_ap=sx[:],
                            num_idxs=cap, num_idxs_reg=cap, elem_size=EMB,
                            sbuf_tokens_per_rank=P, parity_reg=0, out_ap_other=other,
                            single_packet=False)

                def finalize(acc_e, acc_o, e_sl, o_sl, inv_eo, zrep):
                    """mean = (sum replicas) * inv_dst; out = relu(mean + z)."""
                    es = acc_e[0][:, e_sl[0]:e_sl[1], :]
                    os_ = acc_o[0][:, o_sl[0]:o_sl[1], :]
                    for r in range(1, NREP):
                        nc.vector.tensor_tensor(out=es, in0=es,
                                                in1=acc_e[r][:, e_sl[0]:e_sl[1], :],
                                                op=mybir.AluOpType.add)
                        nc.vector.tensor_tensor(out=os_, in0=os_,
                                                in1=acc_o[r][:, o_sl[0]:o_sl[1], :],
                                                op=mybir.AluOpType.add)
                    ive, ivo = inv_eo
                    nc.vector.tensor_tensor(
                        out=es, in0=es,
                        in1=wsb[ive][:, e_sl[0]:e_sl[1], None].to_broadcast(
                            [P, e_sl[1] - e_sl[0], EMB]),
                        op=mybir.AluOpType.mult)
                    nc.vector.tensor_tensor(
                        out=os_, in0=os_,
                        in1=wsb[ivo][:, o_sl[0]:o_sl[1], None].to_broadcast(
                            [P, o_sl[1] - o_sl[0], EMB]),
                        op=mybir.AluOpType.mult)
                    ze, zo = zrep
                    nc.vector.tensor_tensor(out=es, in0=es,
                                            in1=ze[:, e_sl[0]:e_sl[1], :],
                                            op=mybir.AluOpType.add)
                    nc.vector.tensor_tensor(out=os_, in0=os_,
                                            in1=zo[:, o_sl[0]:o_sl[1], :],
                                            op=mybir.AluOpType.add)
                    nc.vector.tensor_scalar_max(out=es, in0=es, scalar1=0.0)
                    nc.vector.tensor_scalar_max(out=os_, in0=os_, scalar1=0.0)
                    return acc_e[0], acc_o[0]

                def transform_wide(res_e, res_o, t0, t1, rwb, bias4, dst_dram):
                    """tiles t0..t1: x=res[t] -> xT -> x@rw(+bias) -> dst rows."""
                    dv = dst_dram.ap()[:, 0:EMB].rearrange("(t p) e -> t p e", p=P)
                    for b0 in range(t0, t1, 4):
                        ng = min(4, t1 - b0)
                        xT4 = workp.tile([EMB, 4 * P], BF16, tag="tt_xT4")
                        for i in range(ng):
                            t = b0 + i
                            buf = res_e if t % 2 == 0 else res_o
                            g = t // 2
                            tp = psum_t.tile([EMB, P], BF16, tag="tt_tp")
                            nc.tensor.transpose(out=tp[:], in_=buf[:, g, :],
                                                identity=ident[:])
                            nc.vector.tensor_copy(out=xT4[:, i * P:(i + 1) * P], in_=tp[:])
                        op_ = psum_t.tile([P, 4, EMB], F32, tag="tt_op")
                        for i in range(ng):
                            nc.tensor.matmul(out=op_[:, i, :],
                                             lhsT=xT4[:, i * P:(i + 1) * P],
                                             rhs=wsb[rwb][:], start=True, stop=True)
                        os_ = workp.tile([P, 4, EMB], BF16, tag="tt_os")
                        if bias4 is not None:
                            nc.vector.tensor_tensor(
                                out=os_[:, :ng, :], in0=op_[:, :ng, :],
                                in1=wsb[bias4][:].rearrange("p (g e) -> p g e", e=EMB)[:, :ng, :],
                                op=mybir.AluOpType.add)
                        else:
                            nc.vector.tensor_copy(out=os_[:, :ng, :], in_=op_[:, :ng, :])
                        for i in range(ng):
                            nc.sync.dma_start(out=dv[b0 + i], in_=os_[:, i, :])

                vt, ct = pr.VshP // P, pr.CshP // P
                hAp = pr.CshP // 2
                gA = hAp // (2 * P)  # 24.5 -> use explicit tile counts below
                # A-half: tiles [0, 49): e-cols [0,25), o-cols [0,24)
                # B-half: tiles [49, 98): e-cols [25,49), o-cols [24,49)
                tA = hAp // P  # 49

                # emission interleave: cons chunk-0 (cv0 can start), var
                # A-half (first half-AllGather input), cons chunk-1, var rest,
                # second half-AG, remaining cons chunks.  All embed relus run
                # on DVE: the Act engine hosts ONLY collectives (it is held
                # for a collective's full duration).
                cb_ = 32768 // RWIDE
                nvb = pr.VshPP // RWIDE
                def emb_c(b0, b1):
                    embed_wide(cons_xT, pr.cons_nf, pr.Cfull,
                               "cons_w1", "cons_b1", "cons_w2", "cons_b2",
                               "ll_w00", y_c0, "lr_w01", "btz_c", z_c0, pr.CshP,
                               blocks=range(b0, b1))
                def emb_v(b0, b1):
                    embed_wide(var_xT, pr.var_nf, pr.VshPP,
                               "var_w1", "var_b1", "var_w2", "var_b2",
                               "ll_w01", y_v0_sh, "lr_w00", "btz_v", z_v0, pr.VshP,
                               blocks=range(b0, b1))
                emb_c(0, cb_)
                emb_v(0, nvb // 2 + 1)          # covers rows [0, VhR) and a bit more
                emb_c(cb_, 2 * cb_)
                ag_act(y_v0_sh[0:pr.VhR, :], y_v0[0:pr.ncores * pr.VhR, :])
                emb_v(nvb // 2 + 1, nvb)
                emb_c(2 * cb_, 3 * cb_)
                ag_act(y_v0_sh[pr.VhR:pr.VshPP, :],
                       y_v0[pr.ncores * pr.VhR:, :])
                emb_c(3 * cb_, pr.Cfull // RWIDE)

                # ---- conv c->v layer 0 -> z_v1
                pr.cv0.y_full = y_c0
                ae, ao = open_acc(vt, "cv0")
                emit_slots(pr.cv0, "c0", lambda r: (ae[r][:], ao[r][:]))
                zrep = load_zrep(z_v0, vt)
                re_, ro_ = finalize(ae, ao, (0, vt // 2), (0, vt // 2),
                                    ("cvi_e", "cvi_o"), zrep)
                transform_wide(re_, ro_, 0, vt, "lr_w10b", "bt10_4", z_v1)

                # ---- conv v->c layer 0, split into dst-halves A/B with
                # pipelined half-AllGathers of y_c1 ([A-region | B-region] layout)
                pr.vcA.y_full = y_v0
                pr.vcB.y_full = y_v0
                ae, ao = open_acc(ct, "vc", epad=1)
                emit_slots(pr.vcA, "va",
                           lambda r: (ae[r][:, 0:25, :], ao[r][:, 0:25, :]))
                zrep = load_zrep(z_c0, ct)
                re_, ro_ = finalize(ae, ao, (0, 25), (0, 24),
                                    ("vci_e", "vci_o"), zrep)
                transform_wide(re_, ro_, 0, tA, "ll_w10b", None, y_c1_sh)
                ag_act(y_c1_sh[0:hAp, :], y_c1[0:pr.ncores * hAp, :])
                emit_slots(pr.vcB, "vb",
                           lambda r: (ao[r][:, 24:49, :], ae[r][:, 25:50, :]))
                re_, ro_ = finalize(ae, ao, (25, 49), (24, 49),
                                    ("vci_e", "vci_o"), zrep)
                transform_wide(re_, ro_, tA, ct, "ll_w10b", None, y_c1_sh)

                # ---- conv c->v layer 1: A-region slots emitted BEFORE the
                # second half-AllGather so they depend only on the first.
                pr.cv1.y_full = y_c1
                ae, ao = open_acc(vt, "cv1")
                na = pr.cv1_a_slots
                emit_slots(pr.cv1, "c1", lambda r: (ae[r][:], ao[r][:]),
                           order=range(na))
                ag_act(y_c1_sh[hAp:pr.CshP, :], y_c1[pr.ncores * hAp:, :])
                emit_slots(pr.cv1, "c1", lambda r: (ae[r][:], ao[r][:]),
                           order=range(na, len(pr.cv1.slots)))
                zrep = load_zrep(z_v1, vt)
                re_, ro_ = finalize(ae, ao, (0, vt // 2), (0, vt // 2),
                                    ("cvi_e", "cvi_o"), zrep)
                ov = out_xv2.ap().rearrange("(t p) d -> p t d", p=P)
                ge = vt // 2
                for buf, par in ((re_, 0), (ro_, 1)):
                    for g0 in range(0, ge, 8):
                        gn = min(8, ge - g0)
                        of = workp.tile([P, 8, EMB], F32, tag="out_f32")
                        nc.vector.tensor_copy(out=of[:, :gn, :], in_=buf[:, g0:g0 + gn, :])
                        nc.sync.dma_start(
                            out=ov[:, 2 * g0 + par:min(2 * (g0 + gn) + par, vt):2, :],
                            in_=of[:, :gn, :])

        nc.compile()
        return nc

    def assemble(self, results):
        out = np.concatenate([results[k]["out_xv2"][: self.Vsh] for k in range(self.ncores)], 0)
        return out



# ---------------------------------------------------------------- entry points

_CACHE = {}


def _get_built(edge_index):
    key = hash(np.asarray(edge_index).tobytes())
    if key not in _CACHE:
        pr = Problem(100000, 200000, 5, 19)
        pr.prep(np.asarray(edge_index))
        _CACHE.clear()
        _CACHE[key] = (pr, pr.build())
    return _CACHE[key]


def kernel(**inputs):
    pr, nc = _get_built(inputs["edge_index"])
    in_maps = pr.in_maps(inputs)
    from concourse.bass_utils import run_bass_kernel_spmd
    res = run_bass_kernel_spmd(nc, in_maps, core_ids=list(range(pr.ncores)))
    return pr.assemble(res.results).astype(np.float32)


def _pjrt_fn(nc, n_cores, nchain=1):
    """Mirror bass2jax.run_bass_via_pjrt but return a reusable jitted fn
    (no donation) plus the input-name layout, for steady-state timing."""
    import jax
    import concourse.mybir as mb
    from concourse import bass2jax
    from concourse.bass2jax import _bass_exec_p, partition_id_tensor, install_neuronx_cc_hook
    from jax.sharding import Mesh, PartitionSpec
    from jax.experimental.shard_map import shard_map
    install_neuronx_cc_hook()
    partition_name = nc.partition_id_tensor.name if nc.partition_id_tensor else None
    in_names, out_names, out_avals, zero_outs = [], [], [], []
    for alloc in nc.m.functions[0].allocations:
        if not isinstance(alloc, mb.MemoryLocationSet):
            continue
        name = alloc.memorylocations[0].name
        if alloc.kind == "ExternalInput":
            if name != partition_name:
                in_names.append(name)
        elif alloc.kind == "ExternalOutput":
            out_names.append(name)
            shape = tuple(alloc.tensor_shape)
            dtype = mb.dt.np(alloc.dtype)
            out_avals.append(jax.core.ShapedArray(shape, dtype))
            zero_outs.append(np.zeros(shape, dtype))
    n_params = len(in_names)
    all_names = in_names + out_names
    if partition_name is not None:
        all_names_full = all_names + [partition_name]
    def _body(*args):
        params = list(args[:n_params])
        outs = tuple(args[n_params:])
        for _ in range(nchain):
            operands = params + list(outs)
            if partition_name is not None:
                operands.append(partition_id_tensor())
            outs = _bass_exec_p.bind(
                *operands, out_avals=tuple(out_avals),
                in_names=tuple(all_names if partition_name is None else all_names + [partition_name]),
                out_names=tuple(out_names), lowering_input_output_aliases=(),
                sim_require_finite=False, sim_require_nnan=False, nc=nc)
        return tuple(outs)
    devices = jax.devices()[:n_cores]
    mesh = Mesh(np.asarray(devices), ("core",))
    in_specs = (PartitionSpec("core"),) * (n_params + len(out_names))
    out_specs = (PartitionSpec("core"),) * len(out_names)
    fn = jax.jit(shard_map(_body, mesh=mesh, in_specs=in_specs, out_specs=out_specs,
                           check_rep=False), keep_unused=True)
    return fn, in_names, out_names, zero_outs


def run_timed(inputs, iters=4, nchain=6):
    """Returns (full_output, dict with per-exec estimate)."""
    import jax, time
    pr, nc = _get_built(inputs["edge_index"])
    in_maps = pr.in_maps(inputs)
    fn1, in_names, out_names, zero_outs = _pjrt_fn(nc, pr.ncores, nchain=1)
    concat_in = [np.concatenate([np.asarray(in_maps[c][n]) for c in range(pr.ncores)], 0)
                 for n in in_names]
    concat_zero = [np.zeros((pr.ncores * z.shape[0],) + z.shape[1:], z.dtype) for z in zero_outs]
    dev_args = [jax.device_put(a) for a in concat_in + concat_zero]
    out = fn1(*dev_args)
    jax.block_until_ready(out)
    t1s = []
    for _ in range(iters):
        t0 = time.perf_counter()
        out = fn1(*dev_args)
        jax.block_until_ready(out)
        t1s.append(time.perf_counter() - t0)
    times = {"t1": t1s, "tN": t1s, "nchain": 1, "per_exec_s": min(t1s)}
    arrs = [np.asarray(o) for o in out]
    results = []
    for c in range(pr.ncores):
        d = {}
        for i, n in enumerate(out_names):
            per = arrs[i].reshape(pr.ncores, arrs[i].shape[0] // pr.ncores, *arrs[i].shape[1:])
            d[n] = per[c]
        results.append(d)
    return pr.assemble(results).astype(np.float32), times


def predicted_ns(inputs):
    """Cost-model estimate via no-exec CoreSim (core 0)."""
    from concourse.bass_interp import CoreSim
    pr, nc = _get_built(inputs["edge_index"])
    sim = CoreSim(nc, no_exec=True)
    sim.event_loop()
    return sim.time


# revision 20
# speedup vs baseline: 1.1040x; 1.1040x over previous
"""Bipartite GNN encoder (SAGEConv x2 layers) as a Trainium2 Bass/Tile SPMD kernel.

v2 strategy (8 cores):
  - Destination-sharded message passing, linear layers folded into per-node
    transforms (y = x @ ll_w gathered as messages; accum init z = x@lr_w + b).
  - bf16 conv path: y tables stored [N, 128] bf16 (row-padded to 256B so
    dma_gather's 256B-elem constraint is met); messages scaled in bf16;
    dma_scatter_add in bf16 (128B descriptors -> half DMA time) into bf16
    SBUF accumulators.
  - cons embedding is REPLICATED on every core (input table is small) with a
    per-core ROTATED section layout (SPMD program identical; per-core data
    differs) so z_c0 always comes from rotated-section 0.  This removes the
    y_c0 AllGather entirely.
  - var embedding stays sharded + AllGather (y_v0 AG hides under cv0 DMA).
  - Wide (512-row) embed pipeline using host-transposed inputs: no PE
    transposes, ~4x fewer instructions than per-128-tile embeds.
  - Same-dst tokens never share one scatter call (HW RMW race): streams
    sorted by (src_chunk, dst) and strided across >= max_run slots.
  - Layer-1 v->c conv is skipped (its output is unused by the reference).
"""
import sys
sys.path.insert(0, "/opt/trn_rl_repo")
import numpy as np
import concourse.bass as bass
import concourse.bacc as bacc
import concourse.mybir as mybir
import concourse.tile as tile
from concourse.masks import make_identity

F32 = mybir.dt.float32
BF16 = mybir.dt.bfloat16
I16 = mybir.dt.int16
NPBF = mybir.dt.np(mybir.dt.bfloat16)
P = 128
EMB = 64
YW = 128          # padded y-table row width (256B in bf16)
RWIDE = 1024      # wide-embed block rows


def pad_to(n, m):
    return (n + m - 1) // m * m


# ---------------------------------------------------------------- host prep

def pack_idx16(a, cap, pad_val):
    b = np.full(cap, pad_val, np.int64)
    b[: len(a)] = a
    assert b.max() < 32768 and b.min() >= 0
    m = b.astype(np.int16).reshape(cap // 16, 16).T  # token j -> [j%16, j//16]
    return np.tile(m, (8, 1))  # replicate for the 8 q7 cores


def pack_vals(a, cap, dtype):
    b = np.zeros(cap, np.float32)
    b[: len(a)] = a
    return b.reshape(cap // 128, 128).T.astype(dtype).copy()


class ConvPlan:
    """Token stream plan for one conv direction, shared static structure across cores.

    src_gp_list: per-core arrays of global-padded source rows (per-core table
    layouts may differ, e.g. rotated cons tables)."""

    def __init__(self, src_gp_list, dst_g, n_dst, dst_sh_real, dst_sh_pad, src_rows_pad,
                 ncores, chunk, cap_target, edge_mask=None, dst_loc_arr=None,
                 trash=None, chunk_bounds=None):
        self.dst_sh_pad = dst_sh_pad
        owner = dst_g // dst_sh_real
        dst_loc = dst_loc_arr if dst_loc_arr is not None else dst_g - owner * dst_sh_real
        if chunk_bounds is None:
            n_chunks = pad_to(src_rows_pad, chunk) // chunk
            chunk_bounds = [c * chunk for c in range(n_chunks)] + [src_rows_pad]
        n_chunks = len(chunk_bounds) - 1
        cb = np.asarray(chunk_bounds)
        per_core = []  # per core: list over chunks of (gidx_sorted, dloc_sorted, inv_sorted)
        cnt = np.zeros((ncores, n_chunks), np.int64)
        runmax = np.zeros(n_chunks, np.int64)
        for k in range(ncores):
            m = owner == k
            if edge_mask is not None:
                m = m & edge_mask
            src_gp = src_gp_list[k]
            gp_k, dl_k = src_gp[m], dst_loc[m]
            c_k = np.searchsorted(cb, gp_k, side="right") - 1
            order = np.lexsort((dl_k, c_k))
            gp_k, dl_k, c_k = gp_k[order], dl_k[order], c_k[order]
            rows = []
            for c in range(n_chunks):
                mc = c_k == c
                g, d = gp_k[mc] - cb[c], dl_k[mc]
                cnt[k, c] = len(g)
                if len(d):
                    # longest run of equal dst
                    brk = np.flatnonzero(np.diff(d) != 0)
                    edges = np.concatenate([[-1], brk, [len(d) - 1]])
                    runmax[c] = max(runmax[c], np.diff(edges).max())
                rows.append((g, d))
            per_core.append(rows)
        # static slot structure
        self.slots = []  # list of (chunk, cap, src_row_base, src_rows_in_chunk)
        self.nslots_c = []
        for c in range(n_chunks):
            cmax = cnt[:, c].max()
            if cmax == 0:
                self.nslots_c.append(0)
                continue
            ns = int(max(-(-cmax // cap_target), runmax[c], 1))
            cap = pad_to(-(-cmax // ns), 128)
            rows_c = int(cb[c + 1] - cb[c])
            for j in range(ns):
                self.slots.append((c, int(cap), int(cb[c]), rows_c))
            self.nslots_c.append(ns)
        self.tot16 = sum(cap for _, cap, _, _ in self.slots) // 16
        self.tot128 = sum(cap for _, cap, _, _ in self.slots) // 128
        # per-core packed streams
        self.gidx, self.sidx = [], []
        if trash is None:
            trash = dst_sh_pad - 1
        for k in range(ncores):
            gs, ss = [], []
            for c in range(n_chunks):
                ns = self.nslots_c[c]
                if ns == 0:
                    continue
                g, d = per_core[k][c]
                cap = [cp for (cc, cp, _, _) in self.slots if cc == c][0]
                for j in range(ns):
                    gj, dj = g[j::ns], d[j::ns]
                    assert len(gj) <= cap
                    gs.append(pack_idx16(gj, cap, 0))
                    ss.append(pack_idx16(dj, cap, trash))
            self.gidx.append(np.concatenate(gs, axis=1))
            self.sidx.append(np.concatenate(ss, axis=1))


class Problem:
    def __init__(self, n_cons, n_var, cons_nf, var_nf, ncores=8, chunk=32768,
                 cap_target=9216, nrep=2):
        self.ncores, self.chunk, self.cap_target, self.nrep = ncores, chunk, cap_target, nrep
        self.n_cons, self.n_var, self.cons_nf, self.var_nf = n_cons, n_var, cons_nf, var_nf
        assert n_cons % ncores == 0 and n_var % ncores == 0
        self.Csh = n_cons // ncores
        self.Vsh = n_var // ncores
        self.CshP = pad_to(self.Csh + 1, 2 * P)  # +1: last row is scatter-pad trash
        self.VshP = pad_to(self.Vsh + 1, 2 * P)
        self.Cfull = self.CshP * ncores
        self.Vfull = self.VshP * ncores
        self.VshPP = pad_to(self.VshP, 1024)   # wide-embed block padding
        self.CshPP = pad_to(self.CshP, 1024)
        self.VhR = self.VshPP // 2             # v-shard region half (12800)
        self.VfullR = self.VshPP * ncores      # v-table region-layout rows

    # cons-section local rows are "half-mapped": halves of each section are
    # padded independently so the v->c conv + its AllGather can be split into
    # two pipelined halves.  device-local row r = halfmap(l).
    def halfmap(self, l):
        hA, hAp = self.Csh // 2, self.CshP // 2
        return np.where(l < hA, l, hAp + (l - hA))

    def gp_cons_rot(self, idx, k):
        # per-core rotated cons layout: global section s sits at position (s-k)%ncores
        sec = idx // self.Csh
        return ((sec - k) % self.ncores) * self.CshP + self.halfmap(idx % self.Csh)

    def gp_cons_region(self, idx):
        # y_c1 layout: [all A-halves (8 x CshP/2 rows) | all B-halves]
        hA, hAp = self.Csh // 2, self.CshP // 2
        sec, l = idx // self.Csh, idx % self.Csh
        return np.where(l < hA, sec * hAp + l,
                        self.ncores * hAp + sec * hAp + (l - hA))

    def gp_var(self, idx):
        # y_v0 layout: [all A-halves (8 x VhR rows) | all B-halves]
        k, l = idx // self.Vsh, idx % self.Vsh
        return np.where(l < self.VhR, k * self.VhR + l,
                        self.ncores * self.VhR + k * self.VhR + (l - self.VhR))

    def prep(self, edge_index):
        src, dst = np.asarray(edge_index[0]), np.asarray(edge_index[1])
        nc_ = self.ncores
        hA, hAp = self.Csh // 2, self.CshP // 2
        self.cv0 = ConvPlan([self.gp_cons_rot(src, k) for k in range(nc_)],
                            dst, self.n_var, self.Vsh, self.VshP,
                            self.Cfull, nc_, self.chunk, self.cap_target)
        gpc = self.gp_cons_region(src)
        half_rows = nc_ * hAp
        cb = [0, 32768, half_rows, half_rows + 32768, 2 * half_rows]
        self.cv1 = ConvPlan([gpc] * nc_, dst, self.n_var, self.Vsh, self.VshP,
                            self.Cfull, nc_, self.chunk, self.cap_target,
                            chunk_bounds=cb)
        self.cv1_a_slots = sum(self.cv1.nslots_c[:2])
        gpv = self.gp_var(dst)
        cl = src % self.Csh
        hm = self.halfmap(cl)
        vhalf = self.ncores * self.VhR
        vcb = ([0, 32768, 65536, 98304, vhalf, vhalf + 32768, vhalf + 65536,
                vhalf + 98304, 2 * vhalf])
        self.vcA = ConvPlan([gpv] * nc_, src, self.n_cons, self.Csh, self.CshP,
                            self.VfullR, nc_, self.chunk, self.cap_target,
                            edge_mask=cl < hA, dst_loc_arr=hm, trash=hAp - 1,
                            chunk_bounds=vcb)
        self.vcB = ConvPlan([gpv] * nc_, src, self.n_cons, self.Csh, self.CshP,
                            self.VfullR, nc_, self.chunk, self.cap_target,
                            edge_mask=cl >= hA, dst_loc_arr=hm - hAp, trash=hAp - 1,
                            chunk_bounds=vcb)

    # ------------------------------------------------------------ in_maps
    def in_maps(self, inputs):
        ii = {k: np.asarray(v) for k, v in inputs.items()}
        maps = []
        rep4 = lambda b: np.tile(np.repeat(np.asarray(b, np.float32)[None, :], P, 0),
                                 (1, 4))  # [128, 4*64] bias tile
        cons_x = ii["cons_x"]
        var_x = ii["var_x"]
        for k in range(self.ncores):
            cxf = np.zeros((self.Cfull, self.cons_nf), np.float32)
            hm = self.halfmap(np.arange(self.Csh))
            for s in range(self.ncores):
                rot = (s - k) % self.ncores
                cxf[rot * self.CshP + hm] = cons_x[s * self.Csh:(s + 1) * self.Csh]
            vx = np.zeros((self.VshPP, self.var_nf), np.float32)
            vx[: self.Vsh] = var_x[k * self.Vsh:(k + 1) * self.Vsh]
            # per-dst-row 1/deg tables, laid out like the accumulators
            def inv_tiles(deg_full, sh_real, sh_pad, loc_map=None):
                iv = np.ones(sh_pad, np.float32)
                loc = loc_map if loc_map is not None else np.arange(sh_real)
                iv[loc] = 1.0 / np.maximum(deg_full[k * sh_real:(k + 1) * sh_real], 1)
                t = iv.reshape(-1, P).T  # [P, ntiles]
                return (np.ascontiguousarray(t[:, 0::2]).astype(NPBF),
                        np.ascontiguousarray(t[:, 1::2]).astype(NPBF))
            deg_v = np.bincount(np.asarray(ii["edge_index"][1]), minlength=self.n_var)
            deg_c = np.bincount(np.asarray(ii["edge_index"][0]), minlength=self.n_cons)
            cvi_e, cvi_o = inv_tiles(deg_v, self.Vsh, self.VshP)
            vci_e, vci_o = inv_tiles(deg_c, self.Csh, self.CshP,
                                     loc_map=self.halfmap(np.arange(self.Csh)))
            m = {
                "cons_xT": np.ascontiguousarray(cxf.T).astype(NPBF),
                "var_xT": np.ascontiguousarray(vx.T).astype(NPBF),
                # PreNorm folded into the first linear: (x+s)*sc @ w1 + b1
                # == x @ (diag(sc) w1) + (b1 + (s*sc) @ w1)
                "cons_w1": (ii["cons_scale"][:, None] * ii["cons_w1"]).astype(NPBF),
                "cons_b1": (ii["cons_b1"] + (ii["cons_shift"] * ii["cons_scale"])
                            @ ii["cons_w1"]).reshape(-1, 1),
                "cons_w2": ii["cons_w2"].astype(NPBF), "cons_b2": ii["cons_b2"].reshape(-1, 1),
                "var_w1": (ii["var_scale"][:, None] * ii["var_w1"]).astype(NPBF),
                "var_b1": (ii["var_b1"] + (ii["var_shift"] * ii["var_scale"])
                           @ ii["var_w1"]).reshape(-1, 1),
                "var_w2": ii["var_w2"].astype(NPBF), "var_b2": ii["var_b2"].reshape(-1, 1),
                "ll_w00": ii["conv_ll_w"][0, 0].astype(NPBF), "lr_w00": ii["conv_lr_w"][0, 0].astype(NPBF),
                "ll_w01": ii["conv_ll_w"][0, 1].astype(NPBF), "lr_w01": ii["conv_lr_w"][0, 1].astype(NPBF),
                "ll_w10b": ii["conv_ll_w"][1, 0].astype(NPBF),
                "lr_w10b": ii["conv_lr_w"][1, 0].astype(NPBF),
                "btz_c": rep4(ii["conv_ll_b"][0, 1]),   # bias for z_c0 (vc0 init)
                "btz_v": rep4(ii["conv_ll_b"][0, 0]),   # bias for z_v0 (cv0 init)
                "bt10_4": rep4(ii["conv_ll_b"][1, 0]),  # bias for z_v1 (cv1 init)
                "cvi_e": cvi_e, "cvi_o": cvi_o, "vci_e": vci_e, "vci_o": vci_o,
                "c0_gidx": self.cv0.gidx[k], "c0_sidx": self.cv0.sidx[k],
                "c1_gidx": self.cv1.gidx[k], "c1_sidx": self.cv1.sidx[k],
                "va_gidx": self.vcA.gidx[k], "va_sidx": self.vcA.sidx[k],
                "vb_gidx": self.vcB.gidx[k], "vb_sidx": self.vcB.sidx[k],
            }
            maps.append(m)
        return maps

    # ------------------------------------------------------------ kernel
    def build(self, dbg=False):
        pr = self
        nc = bacc.Bacc("TRN2", target_bir_lowering=False, debug=False)
        dp = lambda n, s, d=F32: nc.declare_dram_parameter(n, s, d, isOutput=False)
        cons_xT = dp("cons_xT", [pr.cons_nf, pr.Cfull], BF16)
        var_xT = dp("var_xT", [pr.var_nf, pr.VshPP], BF16)
        w = {}
        for nm, s, d in [("cons_w1", [pr.cons_nf, EMB], BF16), ("cons_b1", [EMB, 1], F32),
                         ("cons_w2", [EMB, EMB], BF16), ("cons_b2", [EMB, 1], F32),
                         ("var_w1", [pr.var_nf, EMB], BF16), ("var_b1", [EMB, 1], F32),
                         ("var_w2", [EMB, EMB], BF16), ("var_b2", [EMB, 1], F32),
                         ("ll_w00", [EMB, EMB], BF16), ("lr_w00", [EMB, EMB], BF16),
                         ("ll_w01", [EMB, EMB], BF16), ("lr_w01", [EMB, EMB], BF16),
                         ("ll_w10b", [EMB, EMB], BF16), ("lr_w10b", [EMB, EMB], BF16),
                         ("btz_c", [P, 8 * EMB], F32), ("btz_v", [P, 8 * EMB], F32),
                         ("bt10_4", [P, 8 * EMB], F32)]:
            w[nm] = dp(nm, s, d)
        streams = {}
        for pre, plan in [("c0", pr.cv0), ("c1", pr.cv1), ("va", pr.vcA), ("vb", pr.vcB)]:
            streams[pre + "_gidx"] = dp(pre + "_gidx", [P, plan.tot16], I16)
            streams[pre + "_sidx"] = dp(pre + "_sidx", [P, plan.tot16], I16)
        for nm, n in [("cvi_e", pr.VshP // 256), ("cvi_o", pr.VshP // 256),
                      ("vci_e", pr.CshP // 256), ("vci_o", pr.CshP // 256)]:
            w[nm] = dp(nm, [P, n], BF16)
        out_xv2 = nc.declare_dram_parameter("out_xv2", [pr.VshP, EMB], F32, isOutput=True)

        y_c0 = nc.dram_tensor("y_c0", [pr.Cfull, YW], BF16)
        y_v0_sh = nc.dram_tensor("y_v0_sh", [pr.VshPP, YW], BF16)
        y_c1_sh = nc.dram_tensor("y_c1_sh", [pr.CshP, YW], BF16)
        y_v0 = nc.dram_tensor("y_v0", [pr.VfullR, YW], BF16, addr_space="Shared")
        y_c1 = nc.dram_tensor("y_c1", [pr.Cfull, YW], BF16, addr_space="Shared")
        z_v0 = nc.dram_tensor("z_v0", [pr.VshPP, EMB], BF16)
        z_c0 = nc.dram_tensor("z_c0", [pr.CshPP, EMB], BF16)
        z_v1 = nc.dram_tensor("z_v1", [pr.VshPP, EMB], BF16)

        RG = [list(range(pr.ncores))]
        NREP = pr.nrep

        with tile.TileContext(nc) as tc:
            with tc.tile_pool(name="const", bufs=1) as constp, \
                 tc.tile_pool(name="work", bufs=3) as workp, \
                 tc.tile_pool(name="msg", bufs=2) as msgp, \
                 tc.tile_pool(name="accp", bufs=1) as accp, \
                 tc.tile_pool(name="psum_e", bufs=1, space="PSUM") as psum_e, \
                 tc.tile_pool(name="psum_t", bufs=1, space="PSUM") as psum_t:

                def ag_act(in_ap, out_ap):
                    bass.BassGpSimd.collective_compute(
                        nc.scalar, "AllGather", mybir.AluOpType.bypass,
                        ins=[in_ap], outs=[out_ap], replica_groups=RG)

                ident = constp.tile([P, P], BF16)
                make_identity(nc, ident[:])
                wsb = {}
                for nm in w:
                    t = constp.tile(list(w[nm].shape), w[nm].dtype, tag=f"w_{nm}",
                                    name=f"wsb_{nm}")
                    nc.sync.dma_start(out=t[:], in_=w[nm][:])
                    wsb[nm] = t

                def embed_wide(xT_dram, nf, nrows, w1, b1, w2, b2,
                               yw, y_dst, zw, zbias4, z_dst, z_rows,
                               blocks=None, use_act=True):
                    """Wide embed: per 1024-row block, MLP + y/z transforms.

                    xT_dram: [nf, nrows] host-transposed input. y_dst: [nrows, YW]
                    bf16 (cols 0:EMB written). z written for rows < z_rows.
                    use_act=False runs the two relus on DVE (bias-add + max) so
                    the Activation engine stays free for collectives."""
                    G = RWIDE // P
                    nblk = -(-nrows // RWIDE)
                    yv = y_dst.ap()[:, 0:EMB].rearrange("(b i p) e -> b p i e", p=P, i=G)
                    zv = z_dst.ap().rearrange("(b i p) e -> b p i e", p=P, i=G)
                    for bi in (blocks if blocks is not None else range(nblk)):
                        rb = bi * RWIDE
                        xin = workp.tile([nf, RWIDE], BF16, tag="e_xin")
                        nc.sync.dma_start(out=xin[:], in_=xT_dram[:, rb:rb + RWIDE])
                        h1 = psum_e.tile([EMB, RWIDE], F32, tag="e_h1")
                        for s in range(RWIDE // 512):
                            nc.tensor.matmul(out=h1[:, s * 512:(s + 1) * 512],
                                             lhsT=wsb[w1][:],
                                             rhs=xin[:, s * 512:(s + 1) * 512],
                                             start=True, stop=True)
                        h1s = workp.tile([EMB, RWIDE], BF16, tag="e_h1s")
                        nc.vector.tensor_scalar(
                            out=h1s[:], in0=h1[:], scalar1=wsb[b1][:], scalar2=0.0,
                            op0=mybir.AluOpType.add, op1=mybir.AluOpType.max)
                        h2 = psum_e.tile([EMB, RWIDE], F32, tag="e_h2")
                        for s in range(RWIDE // 512):
                            nc.tensor.matmul(out=h2[:, s * 512:(s + 1) * 512],
                                             lhsT=wsb[w2][:],
                                             rhs=h1s[:, s * 512:(s + 1) * 512],
                                             start=True, stop=True)
                        xT = workp.tile([EMB, RWIDE], BF16, tag="e_xT")
                        nc.vector.tensor_scalar(
                            out=xT[:], in0=h2[:], scalar1=wsb[b2][:], scalar2=0.0,
                            op0=mybir.AluOpType.add, op1=mybir.AluOpType.max)
                        yp = psum_e.tile([P, G, EMB], F32, tag="e_yp")
                        for i in range(G):
                            nc.tensor.matmul(out=yp[:, i, :],
                                             lhsT=xT[:, i * P:(i + 1) * P],
                                             rhs=wsb[yw][:], start=True, stop=True)
                        ys = workp.tile([P, G, EMB], BF16, tag="e_ys")
                        nc.vector.tensor_copy(out=ys[:], in_=yp[:])
                        nc.sync.dma_start(out=yv[bi], in_=ys[:])
                        zg = min(G, max(0, (z_rows - rb) // P))
                        if zg > 0:
                            zp = psum_e.tile([P, G, EMB], F32, tag="e_zp")
                            for i in range(zg):
                                nc.tensor.matmul(out=zp[:, i, :],
                                                 lhsT=xT[:, i * P:(i + 1) * P],
                                                 rhs=wsb[zw][:], start=True, stop=True)
                            zs = workp.tile([P, G, EMB], BF16, tag="e_zs")
                            nc.vector.tensor_tensor(
                                out=zs[:, :zg, :], in0=zp[:, :zg, :],
                                in1=wsb[zbias4][:].rearrange("p (g e) -> p g e", e=EMB)[:, :zg, :],
                                op=mybir.AluOpType.add)
                            nc.sync.dma_start(out=zv[bi][:, :zg, :], in_=zs[:, :zg, :])

                def open_acc(ntiles, tagp, epad=0):
                    """Alloc e/o accumulators (+NREP replicas), zeroed.
                    z init is added at finalize (after the 1/deg multiply)."""
                    ge = -(-ntiles // 2)
                    go = ntiles // 2
                    acc_e = [accp.tile([P, ge + epad, EMB], BF16, tag=f"acc_e{r}",
                                       name=f"{tagp}_acc_e{r}") for r in range(NREP)]
                    acc_o = [accp.tile([P, go, EMB], BF16, tag=f"acc_o{r}",
                                       name=f"{tagp}_acc_o{r}") for r in range(NREP)]
                    for r in range(NREP):
                        nc.vector.memset(acc_e[r][:], 0.0)
                        nc.vector.memset(acc_o[r][:], 0.0)
                    return acc_e, acc_o

                def load_zrep(z_dram, ntiles):
                    zv = z_dram.ap().rearrange("(t p) d -> p t d", p=P)
                    ze = accp.tile([P, -(-ntiles // 2), EMB], BF16, tag="zrep_e")
                    zo = accp.tile([P, ntiles // 2, EMB], BF16, tag="zrep_o")
                    nc.sync.dma_start(out=ze[:], in_=zv[:, 0:ntiles:2, :])
                    nc.sync.dma_start(out=zo[:], in_=zv[:, 1:ntiles:2, :])
                    return ze, zo

                def emit_slots(plan, pre, own_fn, order=None):
                    """own_fn(r) -> (own_ap, other_ap) scatter targets.
                    Messages accumulate UNSCALED; 1/deg applied at finalize."""
                    gidx_d = streams[pre + "_gidx"]
                    sidx_d = streams[pre + "_sidx"]
                    offs, o16 = [], 0
                    for (c, cap, base, rows_c) in plan.slots:
                        offs.append(o16)
                        o16 += cap // 16
                    idxs = order if order is not None else range(len(plan.slots))
                    for si_, i in enumerate(idxs):
                        c, cap, base, rows_c = plan.slots[i]
                        f16 = offs[i]
                        gi = workp.tile([P, cap // 16], I16, tag="cv_gi")
                        sx = workp.tile([P, cap // 16], I16, tag="cv_si")
                        nc.sync.dma_start(out=gi[:], in_=gidx_d[:, f16:f16 + cap // 16])
                        nc.sync.dma_start(out=sx[:], in_=sidx_d[:, f16:f16 + cap // 16])
                        msgs = msgp.tile([P, cap // 128, YW], BF16, tag="cv_msg")
                        nc.gpsimd.dma_gather(
                            out_ap=msgs[:], in_ap=plan.y_full[base:base + rows_c, :],
                            idxs_ap=gi[:], num_idxs=cap, num_idxs_reg=cap, elem_size=YW,
                            single_packet=False)
                        sc = msgp.tile([P, cap // 128, EMB], BF16, tag="cv_sc")
                        nc.vector.tensor_copy(out=sc[:], in_=msgs[:, :, 0:EMB])
                        r = si_ % NREP
                        own, other = own_fn(r)
                        nc.gpsimd.dma_scatter_add(
                            out_ap=own, in_ap=sc[:], idxs_ap=sx[:],
                            num_idxs=cap, num_idxs_reg=cap, elem_size=EMB,
                            sbuf_tokens_per_rank=P, parity_reg=0, out_ap_other=other,
                            single_packet=False)

                def finalize(acc_e, acc_o, e_sl, o_sl, inv_eo, zrep):
                    """mean = (sum replicas) * inv_dst; out = relu(mean + z)."""
                    es = acc_e[0][:, e_sl[0]:e_sl[1], :]
                    os_ = acc_o[0][:, o_sl[0]:o_sl[1], :]
                    for r in range(1, NREP):
                        nc.vector.tensor_tensor(out=es, in0=es,
                                                in1=acc_e[r][:, e_sl[0]:e_sl[1], :],
                                                op=mybir.AluOpType.add)
                        nc.vector.tensor_tensor(out=os_, in0=os_,
                                                in1=acc_o[r][:, o_sl[0]:o_sl[1], :],
                                                op=mybir.AluOpType.add)
                    ive, ivo = inv_eo
                    nc.vector.tensor_tensor(
                        out=es, in0=es,
                        in1=wsb[ive][:, e_sl[0]:e_sl[1], None].to_broadcast(
                            [P, e_sl[1] - e_sl[0], EMB]),
                        op=mybir.AluOpType.mult)
                    nc.vector.tensor_tensor(
                        out=os_, in0=os_,
                        in1=wsb[ivo][:, o_sl[0]:o_sl[1], None].to_broadcast(
                            [P, o_sl[1] - o_sl[0], EMB]),
                        op=mybir.AluOpType.mult)
                    ze, zo = zrep
                    nc.vector.tensor_tensor(out=es, in0=es,
                                            in1=ze[:, e_sl[0]:e_sl[1], :],
                                            op=mybir.AluOpType.add)
                    nc.vector.tensor_tensor(out=os_, in0=os_,
                                            in1=zo[:, o_sl[0]:o_sl[1], :],
                                            op=mybir.AluOpType.add)
                    nc.vector.tensor_scalar_max(out=es, in0=es, scalar1=0.0)
                    nc.vector.tensor_scalar_max(out=os_, in0=os_, scalar1=0.0)
                    return acc_e[0], acc_o[0]

                def transform_wide(res_e, res_o, t0, t1, rwb, bias4, dst_dram):
                    """tiles t0..t1: x=res[t] -> xT -> x@rw(+bias) -> dst rows."""
                    dv = dst_dram.ap()[:, 0:EMB].rearrange("(t p) e -> t p e", p=P)
                    for b0 in range(t0, t1, 4):
                        ng = min(4, t1 - b0)
                        xT4 = workp.tile([EMB, 4 * P], BF16, tag="tt_xT4")
                        for i in range(ng):
                            t = b0 + i
                            buf = res_e if t % 2 == 0 else res_o
                            g = t // 2
                            tp = psum_t.tile([EMB, P], BF16, tag="tt_tp")
                            nc.tensor.transpose(out=tp[:], in_=buf[:, g, :],
                                                identity=ident[:])
                            nc.vector.tensor_copy(out=xT4[:, i * P:(i + 1) * P], in_=tp[:])
                        op_ = psum_t.tile([P, 4, EMB], F32, tag="tt_op")
                        for i in range(ng):
                            nc.tensor.matmul(out=op_[:, i, :],
                                             lhsT=xT4[:, i * P:(i + 1) * P],
                                             rhs=wsb[rwb][:], start=True, stop=True)
                        os_ = workp.tile([P, 4, EMB], BF16, tag="tt_os")
                        if bias4 is not None:
                            nc.vector.tensor_tensor(
                                out=os_[:, :ng, :], in0=op_[:, :ng, :],
                                in1=wsb[bias4][:].rearrange("p (g e) -> p g e", e=EMB)[:, :ng, :],
                                op=mybir.AluOpType.add)
                        else:
                            nc.vector.tensor_copy(out=os_[:, :ng, :], in_=op_[:, :ng, :])
                        for i in range(ng):
                            nc.sync.dma_start(out=dv[b0 + i], in_=os_[:, i, :])

                vt, ct = pr.VshP // P, pr.CshP // P
                hAp = pr.CshP // 2
                gA = hAp // (2 * P)  # 24.5 -> use explicit tile counts below
                # A-half: tiles [0, 49): e-cols [0,25), o-cols [0,24)
                # B-half: tiles [49, 98): e-cols [25,49), o-cols [24,49)
                tA = hAp // P  # 49

                # emission interleave: cons chunk-0 (cv0 can start), var
                # A-half (first half-AllGather input), cons chunk-1, var rest,
                # second half-AG, remaining cons chunks.  All embed relus run
                # on DVE: the Act engine hosts ONLY collectives (it is held
                # for a collective's full duration).
                cb_ = 32768 // RWIDE
                nvb = pr.VshPP // RWIDE
                def emb_c(b0, b1):
                    embed_wide(cons_xT, pr.cons_nf, pr.Cfull,
                               "cons_w1", "cons_b1", "cons_w2", "cons_b2",
                               "ll_w00", y_c0, "lr_w01", "btz_c", z_c0, pr.CshP,
                               blocks=range(b0, b1))
                def emb_v(b0, b1):
                    embed_wide(var_xT, pr.var_nf, pr.VshPP,
                               "var_w1", "var_b1", "var_w2", "var_b2",
                               "ll_w01", y_v0_sh, "lr_w00", "btz_v", z_v0, pr.VshP,
                               blocks=range(b0, b1))
                emb_c(0, cb_)
                emb_v(0, nvb // 2 + 1)          # covers rows [0, VhR) and a bit more
                # cv0 slots interleaved chunk-by-chunk with the embed blocks
                # producing that chunk, so every engine queue pipelines.
                pr.cv0.y_full = y_c0
                ae, ao = open_acc(vt, "cv0")
                c0_slots = [[i for i, s in enumerate(pr.cv0.slots) if s[0] == c]
                            for c in range(len(pr.cv0.nslots_c))]
                emit_slots(pr.cv0, "c0", lambda r: (ae[r][:], ao[r][:]),
                           order=c0_slots[0])
                ag_act(y_v0_sh[0:pr.VhR, :], y_v0[0:pr.ncores * pr.VhR, :])
                emb_c(cb_, 2 * cb_)
                emit_slots(pr.cv0, "c0", lambda r: (ae[r][:], ao[r][:]),
                           order=c0_slots[1])
                emb_v(nvb // 2 + 1, nvb)
                ag_act(y_v0_sh[pr.VhR:pr.VshPP, :],
                       y_v0[pr.ncores * pr.VhR:, :])
                emb_c(2 * cb_, 3 * cb_)
                emit_slots(pr.cv0, "c0", lambda r: (ae[r][:], ao[r][:]),
                           order=c0_slots[2])
                emb_c(3 * cb_, pr.Cfull // RWIDE)
                emit_slots(pr.cv0, "c0", lambda r: (ae[r][:], ao[r][:]),
                           order=c0_slots[3])
                zrep = load_zrep(z_v0, vt)
                re_, ro_ = finalize(ae, ao, (0, vt // 2), (0, vt // 2),
                                    ("cvi_e", "cvi_o"), zrep)
                transform_wide(re_, ro_, 0, vt, "lr_w10b", "bt10_4", z_v1)

                # ---- conv v->c layer 0, split into dst-halves A/B with
                # pipelined half-AllGathers of y_c1 ([A-region | B-region] layout)
                pr.vcA.y_full = y_v0
                pr.vcB.y_full = y_v0
                ae, ao = open_acc(ct, "vc", epad=1)
                emit_slots(pr.vcA, "va",
                           lambda r: (ae[r][:, 0:25, :], ao[r][:, 0:25, :]))
                zrep = load_zrep(z_c0, ct)
                re_, ro_ = finalize(ae, ao, (0, 25), (0, 24),
                                    ("vci_e", "vci_o"), zrep)
                transform_wide(re_, ro_, 0, tA, "ll_w10b", None, y_c1_sh)
                ag_act(y_c1_sh[0:hAp, :], y_c1[0:pr.ncores * hAp, :])
                emit_slots(pr.vcB, "vb",
                           lambda r: (ao[r][:, 24:49, :], ae[r][:, 25:50, :]))
                re_, ro_ = finalize(ae, ao, (25, 49), (24, 49),
                                    ("vci_e", "vci_o"), zrep)
                transform_wide(re_, ro_, tA, ct, "ll_w10b", None, y_c1_sh)

                # ---- conv c->v layer 1: A-region slots emitted BEFORE the
                # second half-AllGather so they depend only on the first.
                pr.cv1.y_full = y_c1
                ae, ao = open_acc(vt, "cv1")
                na = pr.cv1_a_slots
                emit_slots(pr.cv1, "c1", lambda r: (ae[r][:], ao[r][:]),
                           order=range(na))
                ag_act(y_c1_sh[hAp:pr.CshP, :], y_c1[pr.ncores * hAp:, :])
                emit_slots(pr.cv1, "c1", lambda r: (ae[r][:], ao[r][:]),
                           order=range(na, len(pr.cv1.slots)))
                zrep = load_zrep(z_v1, vt)
                re_, ro_ = finalize(ae, ao, (0, vt // 2), (0, vt // 2),
                                    ("cvi_e", "cvi_o"), zrep)
                ov = out_xv2.ap().rearrange("(t p) d -> p t d", p=P)
                ge = vt // 2
                for buf, par in ((re_, 0), (ro_, 1)):
                    for g0 in range(0, ge, 8):
                        gn = min(8, ge - g0)
                        of = workp.tile([P, 8, EMB], F32, tag="out_f32")
                        nc.vector.tensor_copy(out=of[:, :gn, :], in_=buf[:, g0:g0 + gn, :])
                        nc.sync.dma_start(
                            out=ov[:, 2 * g0 + par:min(2 * (g0 + gn) + par, vt):2, :],
                            in_=of[:, :gn, :])

        nc.compile()
        return nc

    def assemble(self, results):
        out = np.concatenate([results[k]["out_xv2"][: self.Vsh] for k in range(self.ncores)], 0)
        return out



# ---------------------------------------------------------------- entry points

_CACHE = {}


def _get_built(edge_index):
    key = hash(np.asarray(edge_index).tobytes())
    if key not in _CACHE:
        pr = Problem(100000, 200000, 5, 19)
        pr.prep(np.asarray(edge_index))
        _CACHE.clear()
        _CACHE[key] = (pr, pr.build())
    return _CACHE[key]


def kernel(**inputs):
    pr, nc = _get_built(inputs["edge_index"])
    in_maps = pr.in_maps(inputs)
    from concourse.bass_utils import run_bass_kernel_spmd
    res = run_bass_kernel_spmd(nc, in_maps, core_ids=list(range(pr.ncores)))
    return pr.assemble(res.results).astype(np.float32)


def _pjrt_fn(nc, n_cores, nchain=1):
    """Mirror bass2jax.run_bass_via_pjrt but return a reusable jitted fn
    (no donation) plus the input-name layout, for steady-state timing."""
    import jax
    import concourse.mybir as mb
    from concourse import bass2jax
    from concourse.bass2jax import _bass_exec_p, partition_id_tensor, install_neuronx_cc_hook
    from jax.sharding import Mesh, PartitionSpec
    from jax.experimental.shard_map import shard_map
    install_neuronx_cc_hook()
    partition_name = nc.partition_id_tensor.name if nc.partition_id_tensor else None
    in_names, out_names, out_avals, zero_outs = [], [], [], []
    for alloc in nc.m.functions[0].allocations:
        if not isinstance(alloc, mb.MemoryLocationSet):
            continue
        name = alloc.memorylocations[0].name
        if alloc.kind == "ExternalInput":
            if name != partition_name:
                in_names.append(name)
        elif alloc.kind == "ExternalOutput":
            out_names.append(name)
            shape = tuple(alloc.tensor_shape)
            dtype = mb.dt.np(alloc.dtype)
            out_avals.append(jax.core.ShapedArray(shape, dtype))
            zero_outs.append(np.zeros(shape, dtype))
    n_params = len(in_names)
    all_names = in_names + out_names
    if partition_name is not None:
        all_names_full = all_names + [partition_name]
    def _body(*args):
        params = list(args[:n_params])
        outs = tuple(args[n_params:])
        for _ in range(nchain):
            operands = params + list(outs)
            if partition_name is not None:
                operands.append(partition_id_tensor())
            outs = _bass_exec_p.bind(
                *operands, out_avals=tuple(out_avals),
                in_names=tuple(all_names if partition_name is None else all_names + [partition_name]),
                out_names=tuple(out_names), lowering_input_output_aliases=(),
                sim_require_finite=False, sim_require_nnan=False, nc=nc)
        return tuple(outs)
    devices = jax.devices()[:n_cores]
    mesh = Mesh(np.asarray(devices), ("core",))
    in_specs = (PartitionSpec("core"),) * (n_params + len(out_names))
    out_specs = (PartitionSpec("core"),) * len(out_names)
    fn = jax.jit(shard_map(_body, mesh=mesh, in_specs=in_specs, out_specs=out_specs,
                           check_rep=False), keep_unused=True)
    return fn, in_names, out_names, zero_outs


def run_timed(inputs, iters=4, nchain=6):
    """Returns (full_output, dict with per-exec estimate)."""
    import jax, time
    pr, nc = _get_built(inputs["edge_index"])
    in_maps = pr.in_maps(inputs)
    fn1, in_names, out_names, zero_outs = _pjrt_fn(nc, pr.ncores, nchain=1)
    concat_in = [np.concatenate([np.asarray(in_maps[c][n]) for c in range(pr.ncores)], 0)
                 for n in in_names]
    concat_zero = [np.zeros((pr.ncores * z.shape[0],) + z.shape[1:], z.dtype) for z in zero_outs]
    dev_args = [jax.device_put(a) for a in concat_in + concat_zero]
    out = fn1(*dev_args)
    jax.block_until_ready(out)
    t1s = []
    for _ in range(iters):
        t0 = time.perf_counter()
        out = fn1(*dev_args)
        jax.block_until_ready(out)
        t1s.append(time.perf_counter() - t0)
    times = {"t1": t1s, "tN": t1s, "nchain": 1, "per_exec_s": min(t1s)}
    arrs = [np.asarray(o) for o in out]
    results = []
    for c in range(pr.ncores):
        d = {}
        for i, n in enumerate(out_names):
            per = arrs[i].reshape(pr.ncores, arrs[i].shape[0] // pr.ncores, *arrs[i].shape[1:])
            d[n] = per[c]
        results.append(d)
    return pr.assemble(results).astype(np.float32), times


def predicted_ns(inputs):
    """Cost-model estimate via no-exec CoreSim (core 0)."""
    from concourse.bass_interp import CoreSim
    pr, nc = _get_built(inputs["edge_index"])
    sim = CoreSim(nc, no_exec=True)
    sim.event_loop()
    return sim.time


# revision 22
# speedup vs baseline: 1.1057x; 1.0015x over previous
"""Bipartite GNN encoder (SAGEConv x2 layers) as a Trainium2 Bass/Tile SPMD kernel.

v2 strategy (8 cores):
  - Destination-sharded message passing, linear layers folded into per-node
    transforms (y = x @ ll_w gathered as messages; accum init z = x@lr_w + b).
  - bf16 conv path: y tables stored [N, 128] bf16 (row-padded to 256B so
    dma_gather's 256B-elem constraint is met); messages scaled in bf16;
    dma_scatter_add in bf16 (128B descriptors -> half DMA time) into bf16
    SBUF accumulators.
  - cons embedding is REPLICATED on every core (input table is small) with a
    per-core ROTATED section layout (SPMD program identical; per-core data
    differs) so z_c0 always comes from rotated-section 0.  This removes the
    y_c0 AllGather entirely.
  - var embedding stays sharded + AllGather (y_v0 AG hides under cv0 DMA).
  - Wide (512-row) embed pipeline using host-transposed inputs: no PE
    transposes, ~4x fewer instructions than per-128-tile embeds.
  - Same-dst tokens never share one scatter call (HW RMW race): streams
    sorted by (src_chunk, dst) and strided across >= max_run slots.
  - Layer-1 v->c conv is skipped (its output is unused by the reference).
"""
import sys
sys.path.insert(0, "/opt/trn_rl_repo")
import numpy as np
import concourse.bass as bass
import concourse.bacc as bacc
import concourse.mybir as mybir
import concourse.tile as tile
from concourse.masks import make_identity

F32 = mybir.dt.float32
BF16 = mybir.dt.bfloat16
I16 = mybir.dt.int16
NPBF = mybir.dt.np(mybir.dt.bfloat16)
P = 128
EMB = 64
YW = 128          # padded y-table row width (256B in bf16)
RWIDE = 1024      # wide-embed block rows


def pad_to(n, m):
    return (n + m - 1) // m * m


# ---------------------------------------------------------------- host prep

def pack_idx16(a, cap, pad_val):
    b = np.full(cap, pad_val, np.int64)
    b[: len(a)] = a
    assert b.max() < 32768 and b.min() >= 0
    m = b.astype(np.int16).reshape(cap // 16, 16).T  # token j -> [j%16, j//16]
    return np.tile(m, (8, 1))  # replicate for the 8 q7 cores


def pack_vals(a, cap, dtype):
    b = np.zeros(cap, np.float32)
    b[: len(a)] = a
    return b.reshape(cap // 128, 128).T.astype(dtype).copy()


class ConvPlan:
    """Token stream plan for one conv direction, shared static structure across cores.

    src_gp_list: per-core arrays of global-padded source rows (per-core table
    layouts may differ, e.g. rotated cons tables)."""

    def __init__(self, src_gp_list, dst_g, n_dst, dst_sh_real, dst_sh_pad, src_rows_pad,
                 ncores, chunk, cap_target, edge_mask=None, dst_loc_arr=None,
                 trash=None, chunk_bounds=None):
        self.dst_sh_pad = dst_sh_pad
        owner = dst_g // dst_sh_real
        dst_loc = dst_loc_arr if dst_loc_arr is not None else dst_g - owner * dst_sh_real
        if chunk_bounds is None:
            n_chunks = pad_to(src_rows_pad, chunk) // chunk
            chunk_bounds = [c * chunk for c in range(n_chunks)] + [src_rows_pad]
        n_chunks = len(chunk_bounds) - 1
        cb = np.asarray(chunk_bounds)
        per_core = []  # per core: list over chunks of (gidx_sorted, dloc_sorted, inv_sorted)
        cnt = np.zeros((ncores, n_chunks), np.int64)
        runmax = np.zeros(n_chunks, np.int64)
        for k in range(ncores):
            m = owner == k
            if edge_mask is not None:
                m = m & edge_mask
            src_gp = src_gp_list[k]
            gp_k, dl_k = src_gp[m], dst_loc[m]
            c_k = np.searchsorted(cb, gp_k, side="right") - 1
            order = np.lexsort((dl_k, c_k))
            gp_k, dl_k, c_k = gp_k[order], dl_k[order], c_k[order]
            rows = []
            for c in range(n_chunks):
                mc = c_k == c
                g, d = gp_k[mc] - cb[c], dl_k[mc]
                cnt[k, c] = len(g)
                if len(d):
                    # longest run of equal dst
                    brk = np.flatnonzero(np.diff(d) != 0)
                    edges = np.concatenate([[-1], brk, [len(d) - 1]])
                    runmax[c] = max(runmax[c], np.diff(edges).max())
                rows.append((g, d))
            per_core.append(rows)
        # static slot structure
        self.slots = []  # list of (chunk, cap, src_row_base, src_rows_in_chunk)
        self.nslots_c = []
        for c in range(n_chunks):
            cmax = cnt[:, c].max()
            if cmax == 0:
                self.nslots_c.append(0)
                continue
            ns = int(max(-(-cmax // cap_target), runmax[c], 1))
            cap = pad_to(-(-cmax // ns), 128)
            rows_c = int(cb[c + 1] - cb[c])
            for j in range(ns):
                self.slots.append((c, int(cap), int(cb[c]), rows_c))
            self.nslots_c.append(ns)
        self.tot16 = sum(cap for _, cap, _, _ in self.slots) // 16
        self.tot128 = sum(cap for _, cap, _, _ in self.slots) // 128
        # per-core packed streams
        self.gidx, self.sidx = [], []
        if trash is None:
            trash = dst_sh_pad - 1
        for k in range(ncores):
            gs, ss = [], []
            for c in range(n_chunks):
                ns = self.nslots_c[c]
                if ns == 0:
                    continue
                g, d = per_core[k][c]
                cap = [cp for (cc, cp, _, _) in self.slots if cc == c][0]
                for j in range(ns):
                    gj, dj = g[j::ns], d[j::ns]
                    assert len(gj) <= cap
                    gs.append(pack_idx16(gj, cap, 0))
                    ss.append(pack_idx16(dj, cap, trash))
            self.gidx.append(np.concatenate(gs, axis=1))
            self.sidx.append(np.concatenate(ss, axis=1))


class Problem:
    def __init__(self, n_cons, n_var, cons_nf, var_nf, ncores=8, chunk=32768,
                 cap_target=9216, nrep=2):
        self.ncores, self.chunk, self.cap_target, self.nrep = ncores, chunk, cap_target, nrep
        self.n_cons, self.n_var, self.cons_nf, self.var_nf = n_cons, n_var, cons_nf, var_nf
        assert n_cons % ncores == 0 and n_var % ncores == 0
        self.Csh = n_cons // ncores
        self.Vsh = n_var // ncores
        self.CshP = pad_to(self.Csh + 1, 2 * P)  # +1: last row is scatter-pad trash
        self.VshP = pad_to(self.Vsh + 1, 2 * P)
        self.Cfull = self.CshP * ncores
        self.Vfull = self.VshP * ncores
        self.VshPP = pad_to(self.VshP, 1024)   # wide-embed block padding
        self.CshPP = pad_to(self.CshP, 1024)
        self.VhR = self.VshPP // 2             # v-shard region half (12800)
        self.VfullR = self.VshPP * ncores      # v-table region-layout rows

    # cons-section local rows are "half-mapped": halves of each section are
    # padded independently so the v->c conv + its AllGather can be split into
    # two pipelined halves.  device-local row r = halfmap(l).
    def halfmap(self, l):
        hA, hAp = self.Csh // 2, self.CshP // 2
        return np.where(l < hA, l, hAp + (l - hA))

    def gp_cons_rot(self, idx, k):
        # per-core rotated cons layout: global section s sits at position (s-k)%ncores
        sec = idx // self.Csh
        return ((sec - k) % self.ncores) * self.CshP + self.halfmap(idx % self.Csh)

    def gp_cons_region(self, idx):
        # y_c1 layout: [all A-halves (8 x CshP/2 rows) | all B-halves]
        hA, hAp = self.Csh // 2, self.CshP // 2
        sec, l = idx // self.Csh, idx % self.Csh
        return np.where(l < hA, sec * hAp + l,
                        self.ncores * hAp + sec * hAp + (l - hA))

    def gp_var(self, idx):
        # y_v0 layout: [all A-halves (8 x VhR rows) | all B-halves]
        k, l = idx // self.Vsh, idx % self.Vsh
        return np.where(l < self.VhR, k * self.VhR + l,
                        self.ncores * self.VhR + k * self.VhR + (l - self.VhR))

    def prep(self, edge_index):
        src, dst = np.asarray(edge_index[0]), np.asarray(edge_index[1])
        nc_ = self.ncores
        hA, hAp = self.Csh // 2, self.CshP // 2
        self.cv0 = ConvPlan([self.gp_cons_rot(src, k) for k in range(nc_)],
                            dst, self.n_var, self.Vsh, self.VshP,
                            self.Cfull, nc_, self.chunk, self.cap_target)
        gpc = self.gp_cons_region(src)
        half_rows = nc_ * hAp
        cb = [0, 32768, half_rows, half_rows + 32768, 2 * half_rows]
        self.cv1 = ConvPlan([gpc] * nc_, dst, self.n_var, self.Vsh, self.VshP,
                            self.Cfull, nc_, self.chunk, self.cap_target,
                            chunk_bounds=cb)
        self.cv1_a_slots = sum(self.cv1.nslots_c[:2])
        gpv = self.gp_var(dst)
        cl = src % self.Csh
        hm = self.halfmap(cl)
        vhalf = self.ncores * self.VhR
        vcb = ([0, 32768, 65536, 98304, vhalf, vhalf + 32768, vhalf + 65536,
                vhalf + 98304, 2 * vhalf])
        self.vcA = ConvPlan([gpv] * nc_, src, self.n_cons, self.Csh, self.CshP,
                            self.VfullR, nc_, self.chunk, self.cap_target,
                            edge_mask=cl < hA, dst_loc_arr=hm, trash=hAp - 1,
                            chunk_bounds=vcb)
        self.vcB = ConvPlan([gpv] * nc_, src, self.n_cons, self.Csh, self.CshP,
                            self.VfullR, nc_, self.chunk, self.cap_target,
                            edge_mask=cl >= hA, dst_loc_arr=hm - hAp, trash=hAp - 1,
                            chunk_bounds=vcb)

    # ------------------------------------------------------------ in_maps
    def in_maps(self, inputs):
        ii = {k: np.asarray(v) for k, v in inputs.items()}
        maps = []
        rep4 = lambda b: np.tile(np.repeat(np.asarray(b, np.float32)[None, :], P, 0),
                                 (1, 4))  # [128, 4*64] bias tile
        cons_x = ii["cons_x"]
        var_x = ii["var_x"]
        for k in range(self.ncores):
            cxf = np.zeros((self.Cfull, self.cons_nf), np.float32)
            hm = self.halfmap(np.arange(self.Csh))
            for s in range(self.ncores):
                rot = (s - k) % self.ncores
                cxf[rot * self.CshP + hm] = cons_x[s * self.Csh:(s + 1) * self.Csh]
            vx = np.zeros((self.VshPP, self.var_nf), np.float32)
            vx[: self.Vsh] = var_x[k * self.Vsh:(k + 1) * self.Vsh]
            # per-dst-row 1/deg tables, laid out like the accumulators
            def inv_tiles(deg_full, sh_real, sh_pad, loc_map=None):
                iv = np.ones(sh_pad, np.float32)
                loc = loc_map if loc_map is not None else np.arange(sh_real)
                iv[loc] = 1.0 / np.maximum(deg_full[k * sh_real:(k + 1) * sh_real], 1)
                t = iv.reshape(-1, P).T  # [P, ntiles]
                return (np.ascontiguousarray(t[:, 0::2]).astype(NPBF),
                        np.ascontiguousarray(t[:, 1::2]).astype(NPBF))
            deg_v = np.bincount(np.asarray(ii["edge_index"][1]), minlength=self.n_var)
            deg_c = np.bincount(np.asarray(ii["edge_index"][0]), minlength=self.n_cons)
            cvi_e, cvi_o = inv_tiles(deg_v, self.Vsh, self.VshP)
            vci_e, vci_o = inv_tiles(deg_c, self.Csh, self.CshP,
                                     loc_map=self.halfmap(np.arange(self.Csh)))
            m = {
                "cons_xT": np.ascontiguousarray(cxf.T).astype(NPBF),
                "var_xT": np.ascontiguousarray(vx.T).astype(NPBF),
                # PreNorm folded into the first linear: (x+s)*sc @ w1 + b1
                # == x @ (diag(sc) w1) + (b1 + (s*sc) @ w1)
                "cons_w1": (ii["cons_scale"][:, None] * ii["cons_w1"]).astype(NPBF),
                "cons_b1": (ii["cons_b1"] + (ii["cons_shift"] * ii["cons_scale"])
                            @ ii["cons_w1"]).reshape(-1, 1),
                "cons_w2": ii["cons_w2"].astype(NPBF), "cons_b2": ii["cons_b2"].reshape(-1, 1),
                "var_w1": (ii["var_scale"][:, None] * ii["var_w1"]).astype(NPBF),
                "var_b1": (ii["var_b1"] + (ii["var_shift"] * ii["var_scale"])
                           @ ii["var_w1"]).reshape(-1, 1),
                "var_w2": ii["var_w2"].astype(NPBF), "var_b2": ii["var_b2"].reshape(-1, 1),
                "ll_w00": ii["conv_ll_w"][0, 0].astype(NPBF), "lr_w00": ii["conv_lr_w"][0, 0].astype(NPBF),
                "ll_w01": ii["conv_ll_w"][0, 1].astype(NPBF), "lr_w01": ii["conv_lr_w"][0, 1].astype(NPBF),
                "ll_w10b": ii["conv_ll_w"][1, 0].astype(NPBF),
                "lr_w10b": ii["conv_lr_w"][1, 0].astype(NPBF),
                "btz_c": rep4(ii["conv_ll_b"][0, 1]),   # bias for z_c0 (vc0 init)
                "btz_v": rep4(ii["conv_ll_b"][0, 0]),   # bias for z_v0 (cv0 init)
                "bt10_4": rep4(ii["conv_ll_b"][1, 0]),  # bias for z_v1 (cv1 init)
                "cvi_e": cvi_e, "cvi_o": cvi_o, "vci_e": vci_e, "vci_o": vci_o,
                "c0_gidx": self.cv0.gidx[k], "c0_sidx": self.cv0.sidx[k],
                "c1_gidx": self.cv1.gidx[k], "c1_sidx": self.cv1.sidx[k],
                "va_gidx": self.vcA.gidx[k], "va_sidx": self.vcA.sidx[k],
                "vb_gidx": self.vcB.gidx[k], "vb_sidx": self.vcB.sidx[k],
            }
            maps.append(m)
        return maps

    # ------------------------------------------------------------ kernel
    def build(self, dbg=False):
        pr = self
        nc = bacc.Bacc("TRN2", target_bir_lowering=False, debug=False)
        dp = lambda n, s, d=F32: nc.declare_dram_parameter(n, s, d, isOutput=False)
        cons_xT = dp("cons_xT", [pr.cons_nf, pr.Cfull], BF16)
        var_xT = dp("var_xT", [pr.var_nf, pr.VshPP], BF16)
        w = {}
        for nm, s, d in [("cons_w1", [pr.cons_nf, EMB], BF16), ("cons_b1", [EMB, 1], F32),
                         ("cons_w2", [EMB, EMB], BF16), ("cons_b2", [EMB, 1], F32),
                         ("var_w1", [pr.var_nf, EMB], BF16), ("var_b1", [EMB, 1], F32),
                         ("var_w2", [EMB, EMB], BF16), ("var_b2", [EMB, 1], F32),
                         ("ll_w00", [EMB, EMB], BF16), ("lr_w00", [EMB, EMB], BF16),
                         ("ll_w01", [EMB, EMB], BF16), ("lr_w01", [EMB, EMB], BF16),
                         ("ll_w10b", [EMB, EMB], BF16), ("lr_w10b", [EMB, EMB], BF16),
                         ("btz_c", [P, 8 * EMB], F32), ("btz_v", [P, 8 * EMB], F32),
                         ("bt10_4", [P, 8 * EMB], F32)]:
            w[nm] = dp(nm, s, d)
        streams = {}
        for pre, plan in [("c0", pr.cv0), ("c1", pr.cv1), ("va", pr.vcA), ("vb", pr.vcB)]:
            streams[pre + "_gidx"] = dp(pre + "_gidx", [P, plan.tot16], I16)
            streams[pre + "_sidx"] = dp(pre + "_sidx", [P, plan.tot16], I16)
        for nm, n in [("cvi_e", pr.VshP // 256), ("cvi_o", pr.VshP // 256),
                      ("vci_e", pr.CshP // 256), ("vci_o", pr.CshP // 256)]:
            w[nm] = dp(nm, [P, n], BF16)
        out_xv2 = nc.declare_dram_parameter("out_xv2", [pr.VshP, EMB], F32, isOutput=True)

        y_c0 = nc.dram_tensor("y_c0", [pr.Cfull, YW], BF16)
        y_v0_sh = nc.dram_tensor("y_v0_sh", [pr.VshPP, YW], BF16)
        y_c1_sh = nc.dram_tensor("y_c1_sh", [pr.CshP, YW], BF16)
        y_v0 = nc.dram_tensor("y_v0", [pr.VfullR, YW], BF16, addr_space="Shared")
        y_c1 = nc.dram_tensor("y_c1", [pr.Cfull, YW], BF16, addr_space="Shared")
        z_v0 = nc.dram_tensor("z_v0", [pr.VshPP, EMB], BF16)
        z_c0 = nc.dram_tensor("z_c0", [pr.CshPP, EMB], BF16)
        z_v1 = nc.dram_tensor("z_v1", [pr.VshPP, EMB], BF16)

        RG = [list(range(pr.ncores))]
        NREP = pr.nrep

        with tile.TileContext(nc) as tc:
            with tc.tile_pool(name="const", bufs=1) as constp, \
                 tc.tile_pool(name="work", bufs=3) as workp, \
                 tc.tile_pool(name="msg", bufs=2) as msgp, \
                 tc.tile_pool(name="accp", bufs=1) as accp, \
                 tc.tile_pool(name="psum_e", bufs=1, space="PSUM") as psum_e, \
                 tc.tile_pool(name="psum_t", bufs=1, space="PSUM") as psum_t:

                def ag_act(in_ap, out_ap):
                    bass.BassGpSimd.collective_compute(
                        nc.scalar, "AllGather", mybir.AluOpType.bypass,
                        ins=[in_ap], outs=[out_ap], replica_groups=RG)

                ident = constp.tile([P, P], BF16)
                make_identity(nc, ident[:])
                wsb = {}
                for nm in w:
                    t = constp.tile(list(w[nm].shape), w[nm].dtype, tag=f"w_{nm}",
                                    name=f"wsb_{nm}")
                    nc.sync.dma_start(out=t[:], in_=w[nm][:])
                    wsb[nm] = t

                def embed_wide(xT_dram, nf, nrows, w1, b1, w2, b2,
                               yw, y_dst, zw, zbias4, z_dst, z_rows,
                               blocks=None, use_act=True):
                    """Wide embed: per 1024-row block, MLP + y/z transforms.

                    xT_dram: [nf, nrows] host-transposed input. y_dst: [nrows, YW]
                    bf16 (cols 0:EMB written). z written for rows < z_rows.
                    use_act=False runs the two relus on DVE (bias-add + max) so
                    the Activation engine stays free for collectives."""
                    G = RWIDE // P
                    nblk = -(-nrows // RWIDE)
                    yv = y_dst.ap()[:, 0:EMB].rearrange("(b i p) e -> b p i e", p=P, i=G)
                    zv = z_dst.ap().rearrange("(b i p) e -> b p i e", p=P, i=G)
                    for bi in (blocks if blocks is not None else range(nblk)):
                        rb = bi * RWIDE
                        xin = workp.tile([nf, RWIDE], BF16, tag="e_xin")
                        nc.sync.dma_start(out=xin[:], in_=xT_dram[:, rb:rb + RWIDE])
                        h1 = psum_e.tile([EMB, RWIDE], F32, tag="e_h1")
                        for s in range(RWIDE // 512):
                            nc.tensor.matmul(out=h1[:, s * 512:(s + 1) * 512],
                                             lhsT=wsb[w1][:],
                                             rhs=xin[:, s * 512:(s + 1) * 512],
                                             start=True, stop=True)
                        h1s = workp.tile([EMB, RWIDE], BF16, tag="e_h1s")
                        nc.vector.tensor_scalar(
                            out=h1s[:], in0=h1[:], scalar1=wsb[b1][:], scalar2=0.0,
                            op0=mybir.AluOpType.add, op1=mybir.AluOpType.max)
                        h2 = psum_e.tile([EMB, RWIDE], F32, tag="e_h2")
                        for s in range(RWIDE // 512):
                            nc.tensor.matmul(out=h2[:, s * 512:(s + 1) * 512],
                                             lhsT=wsb[w2][:],
                                             rhs=h1s[:, s * 512:(s + 1) * 512],
                                             start=True, stop=True)
                        xT = workp.tile([EMB, RWIDE], BF16, tag="e_xT")
                        nc.vector.tensor_scalar(
                            out=xT[:], in0=h2[:], scalar1=wsb[b2][:], scalar2=0.0,
                            op0=mybir.AluOpType.add, op1=mybir.AluOpType.max)
                        yp = psum_e.tile([P, G, EMB], F32, tag="e_yp")
                        for i in range(G):
                            nc.tensor.matmul(out=yp[:, i, :],
                                             lhsT=xT[:, i * P:(i + 1) * P],
                                             rhs=wsb[yw][:], start=True, stop=True)
                        ys = workp.tile([P, G, EMB], BF16, tag="e_ys")
                        nc.vector.tensor_copy(out=ys[:], in_=yp[:])
                        nc.sync.dma_start(out=yv[bi], in_=ys[:])
                        zg = min(G, max(0, (z_rows - rb) // P))
                        if zg > 0:
                            zp = psum_e.tile([P, G, EMB], F32, tag="e_zp")
                            for i in range(zg):
                                nc.tensor.matmul(out=zp[:, i, :],
                                                 lhsT=xT[:, i * P:(i + 1) * P],
                                                 rhs=wsb[zw][:], start=True, stop=True)
                            zs = workp.tile([P, G, EMB], BF16, tag="e_zs")
                            nc.vector.tensor_tensor(
                                out=zs[:, :zg, :], in0=zp[:, :zg, :],
                                in1=wsb[zbias4][:].rearrange("p (g e) -> p g e", e=EMB)[:, :zg, :],
                                op=mybir.AluOpType.add)
                            nc.sync.dma_start(out=zv[bi][:, :zg, :], in_=zs[:, :zg, :])

                def open_acc(ntiles, tagp, epad=0):
                    """Alloc e/o accumulators (+NREP replicas), zeroed.
                    z init is added at finalize (after the 1/deg multiply)."""
                    ge = -(-ntiles // 2)
                    go = ntiles // 2
                    acc_e = [accp.tile([P, ge + epad, EMB], BF16, tag=f"acc_e{r}",
                                       name=f"{tagp}_acc_e{r}") for r in range(NREP)]
                    acc_o = [accp.tile([P, go, EMB], BF16, tag=f"acc_o{r}",
                                       name=f"{tagp}_acc_o{r}") for r in range(NREP)]
                    for r in range(NREP):
                        nc.vector.memset(acc_e[r][:], 0.0)
                        nc.vector.memset(acc_o[r][:], 0.0)
                    return acc_e, acc_o

                def load_zrep(z_dram, ntiles):
                    zv = z_dram.ap().rearrange("(t p) d -> p t d", p=P)
                    ze = accp.tile([P, -(-ntiles // 2), EMB], BF16, tag="zrep_e")
                    zo = accp.tile([P, ntiles // 2, EMB], BF16, tag="zrep_o")
                    nc.sync.dma_start(out=ze[:], in_=zv[:, 0:ntiles:2, :])
                    nc.sync.dma_start(out=zo[:], in_=zv[:, 1:ntiles:2, :])
                    return ze, zo

                def emit_slots(plan, pre, own_fn, order=None):
                    """own_fn(r) -> (own_ap, other_ap) scatter targets.
                    Messages accumulate UNSCALED; 1/deg applied at finalize.
                    Index streams are loaded in one DMA per chunk-group."""
                    gidx_d = streams[pre + "_gidx"]
                    sidx_d = streams[pre + "_sidx"]
                    offs, o16 = [], 0
                    for (c, cap, base, rows_c) in plan.slots:
                        offs.append(o16)
                        o16 += cap // 16
                    idxs = list(order if order is not None else range(len(plan.slots)))
                    # group consecutive slots sharing a chunk (bounded size)
                    groups, gsz = [], 0
                    for i in idxs:
                        cap_i = plan.slots[i][1]
                        if (groups and plan.slots[groups[-1][-1]][0] == plan.slots[i][0]
                                and gsz + cap_i <= 24576):
                            groups[-1].append(i)
                            gsz += cap_i
                        else:
                            groups.append([i])
                            gsz = cap_i
                    si_ = 0
                    for grp in groups:
                        g16 = sum(plan.slots[i][1] // 16 for i in grp)
                        f0 = offs[grp[0]]
                        gia = workp.tile([P, g16], I16, tag="cv_gi", bufs=2)
                        sxa = workp.tile([P, g16], I16, tag="cv_si", bufs=2)
                        nc.sync.dma_start(out=gia[:], in_=gidx_d[:, f0:f0 + g16])
                        nc.sync.dma_start(out=sxa[:], in_=sidx_d[:, f0:f0 + g16])
                        rel = 0
                        for i in grp:
                            c, cap, base, rows_c = plan.slots[i]
                            gi = gia[:, rel:rel + cap // 16]
                            sx = sxa[:, rel:rel + cap // 16]
                            rel += cap // 16
                            msgs = msgp.tile([P, cap // 128, YW], BF16, tag="cv_msg")
                            nc.gpsimd.dma_gather(
                                out_ap=msgs[:], in_ap=plan.y_full[base:base + rows_c, :],
                                idxs_ap=gi, num_idxs=cap, num_idxs_reg=cap, elem_size=YW,
                                single_packet=False)
                            sc = msgp.tile([P, cap // 128, EMB], BF16, tag="cv_sc")
                            nc.vector.tensor_copy(out=sc[:], in_=msgs[:, :, 0:EMB])
                            own, other = own_fn(si_ % NREP)
                            si_ += 1
                            nc.gpsimd.dma_scatter_add(
                                out_ap=own, in_ap=sc[:], idxs_ap=sx,
                                num_idxs=cap, num_idxs_reg=cap, elem_size=EMB,
                                sbuf_tokens_per_rank=P, parity_reg=0, out_ap_other=other,
                                single_packet=False)

                def finalize(acc_e, acc_o, e_sl, o_sl, inv_eo, zrep):
                    """mean = (sum replicas) * inv_dst; out = relu(mean + z)."""
                    es = acc_e[0][:, e_sl[0]:e_sl[1], :]
                    os_ = acc_o[0][:, o_sl[0]:o_sl[1], :]
                    for r in range(1, NREP):
                        nc.vector.tensor_tensor(out=es, in0=es,
                                                in1=acc_e[r][:, e_sl[0]:e_sl[1], :],
                                                op=mybir.AluOpType.add)
                        nc.vector.tensor_tensor(out=os_, in0=os_,
                                                in1=acc_o[r][:, o_sl[0]:o_sl[1], :],
                                                op=mybir.AluOpType.add)
                    ive, ivo = inv_eo
                    nc.vector.tensor_tensor(
                        out=es, in0=es,
                        in1=wsb[ive][:, e_sl[0]:e_sl[1], None].to_broadcast(
                            [P, e_sl[1] - e_sl[0], EMB]),
                        op=mybir.AluOpType.mult)
                    nc.vector.tensor_tensor(
                        out=os_, in0=os_,
                        in1=wsb[ivo][:, o_sl[0]:o_sl[1], None].to_broadcast(
                            [P, o_sl[1] - o_sl[0], EMB]),
                        op=mybir.AluOpType.mult)
                    ze, zo = zrep
                    nc.vector.tensor_tensor(out=es, in0=es,
                                            in1=ze[:, e_sl[0]:e_sl[1], :],
                                            op=mybir.AluOpType.add)
                    nc.vector.tensor_tensor(out=os_, in0=os_,
                                            in1=zo[:, o_sl[0]:o_sl[1], :],
                                            op=mybir.AluOpType.add)
                    nc.vector.tensor_scalar_max(out=es, in0=es, scalar1=0.0)
                    nc.vector.tensor_scalar_max(out=os_, in0=os_, scalar1=0.0)
                    return acc_e[0], acc_o[0]

                def transform_wide(res_e, res_o, t0, t1, rwb, bias4, dst_dram):
                    """tiles t0..t1: x=res[t] -> xT -> x@rw(+bias) -> dst rows."""
                    dv = dst_dram.ap()[:, 0:EMB].rearrange("(t p) e -> t p e", p=P)
                    for b0 in range(t0, t1, 4):
                        ng = min(4, t1 - b0)
                        xT4 = workp.tile([EMB, 4 * P], BF16, tag="tt_xT4")
                        for i in range(ng):
                            t = b0 + i
                            buf = res_e if t % 2 == 0 else res_o
                            g = t // 2
                            tp = psum_t.tile([EMB, P], BF16, tag="tt_tp")
                            nc.tensor.transpose(out=tp[:], in_=buf[:, g, :],
                                                identity=ident[:])
                            nc.vector.tensor_copy(out=xT4[:, i * P:(i + 1) * P], in_=tp[:])
                        op_ = psum_t.tile([P, 4, EMB], F32, tag="tt_op")
                        for i in range(ng):
                            nc.tensor.matmul(out=op_[:, i, :],
                                             lhsT=xT4[:, i * P:(i + 1) * P],
                                             rhs=wsb[rwb][:], start=True, stop=True)
                        os_ = workp.tile([P, 4, EMB], BF16, tag="tt_os")
                        if bias4 is not None:
                            nc.vector.tensor_tensor(
                                out=os_[:, :ng, :], in0=op_[:, :ng, :],
                                in1=wsb[bias4][:].rearrange("p (g e) -> p g e", e=EMB)[:, :ng, :],
                                op=mybir.AluOpType.add)
                        else:
                            nc.vector.tensor_copy(out=os_[:, :ng, :], in_=op_[:, :ng, :])
                        for i in range(ng):
                            nc.sync.dma_start(out=dv[b0 + i], in_=os_[:, i, :])

                vt, ct = pr.VshP // P, pr.CshP // P
                hAp = pr.CshP // 2
                gA = hAp // (2 * P)  # 24.5 -> use explicit tile counts below
                # A-half: tiles [0, 49): e-cols [0,25), o-cols [0,24)
                # B-half: tiles [49, 98): e-cols [25,49), o-cols [24,49)
                tA = hAp // P  # 49

                # emission interleave: cons chunk-0 (cv0 can start), var
                # A-half (first half-AllGather input), cons chunk-1, var rest,
                # second half-AG, remaining cons chunks.  All embed relus run
                # on DVE: the Act engine hosts ONLY collectives (it is held
                # for a collective's full duration).
                cb_ = 32768 // RWIDE
                nvb = pr.VshPP // RWIDE
                def emb_c(b0, b1):
                    embed_wide(cons_xT, pr.cons_nf, pr.Cfull,
                               "cons_w1", "cons_b1", "cons_w2", "cons_b2",
                               "ll_w00", y_c0, "lr_w01", "btz_c", z_c0, pr.CshP,
                               blocks=range(b0, b1))
                def emb_v(b0, b1):
                    embed_wide(var_xT, pr.var_nf, pr.VshPP,
                               "var_w1", "var_b1", "var_w2", "var_b2",
                               "ll_w01", y_v0_sh, "lr_w00", "btz_v", z_v0, pr.VshP,
                               blocks=range(b0, b1))
                emb_c(0, cb_)
                emb_v(0, nvb // 2 + 1)          # covers rows [0, VhR) and a bit more
                # cv0 slots interleaved chunk-by-chunk with the embed blocks
                # producing that chunk, so every engine queue pipelines.
                pr.cv0.y_full = y_c0
                ae, ao = open_acc(vt, "cv0")
                c0_slots = [[i for i, s in enumerate(pr.cv0.slots) if s[0] == c]
                            for c in range(len(pr.cv0.nslots_c))]
                emit_slots(pr.cv0, "c0", lambda r: (ae[r][:], ao[r][:]),
                           order=c0_slots[0])
                ag_act(y_v0_sh[0:pr.VhR, :], y_v0[0:pr.ncores * pr.VhR, :])
                emb_c(cb_, 2 * cb_)
                emit_slots(pr.cv0, "c0", lambda r: (ae[r][:], ao[r][:]),
                           order=c0_slots[1])
                emb_v(nvb // 2 + 1, nvb)
                ag_act(y_v0_sh[pr.VhR:pr.VshPP, :],
                       y_v0[pr.ncores * pr.VhR:, :])
                emb_c(2 * cb_, 3 * cb_)
                emit_slots(pr.cv0, "c0", lambda r: (ae[r][:], ao[r][:]),
                           order=c0_slots[2])
                emb_c(3 * cb_, pr.Cfull // RWIDE)
                emit_slots(pr.cv0, "c0", lambda r: (ae[r][:], ao[r][:]),
                           order=c0_slots[3])
                zrep = load_zrep(z_v0, vt)
                re_, ro_ = finalize(ae, ao, (0, vt // 2), (0, vt // 2),
                                    ("cvi_e", "cvi_o"), zrep)
                transform_wide(re_, ro_, 0, vt, "lr_w10b", "bt10_4", z_v1)

                # ---- conv v->c layer 0, split into dst-halves A/B with
                # pipelined half-AllGathers of y_c1 ([A-region | B-region] layout)
                pr.vcA.y_full = y_v0
                pr.vcB.y_full = y_v0
                ae, ao = open_acc(ct, "vc", epad=1)
                emit_slots(pr.vcA, "va",
                           lambda r: (ae[r][:, 0:25, :], ao[r][:, 0:25, :]))
                zrep = load_zrep(z_c0, ct)
                re_, ro_ = finalize(ae, ao, (0, 25), (0, 24),
                                    ("vci_e", "vci_o"), zrep)
                transform_wide(re_, ro_, 0, tA, "ll_w10b", None, y_c1_sh)
                ag_act(y_c1_sh[0:hAp, :], y_c1[0:pr.ncores * hAp, :])
                emit_slots(pr.vcB, "vb",
                           lambda r: (ao[r][:, 24:49, :], ae[r][:, 25:50, :]))
                re_, ro_ = finalize(ae, ao, (25, 49), (24, 49),
                                    ("vci_e", "vci_o"), zrep)
                transform_wide(re_, ro_, tA, ct, "ll_w10b", None, y_c1_sh)

                # ---- conv c->v layer 1: A-region slots emitted BEFORE the
                # second half-AllGather so they depend only on the first.
                pr.cv1.y_full = y_c1
                ae, ao = open_acc(vt, "cv1")
                na = pr.cv1_a_slots
                emit_slots(pr.cv1, "c1", lambda r: (ae[r][:], ao[r][:]),
                           order=range(na))
                ag_act(y_c1_sh[hAp:pr.CshP, :], y_c1[pr.ncores * hAp:, :])
                emit_slots(pr.cv1, "c1", lambda r: (ae[r][:], ao[r][:]),
                           order=range(na, len(pr.cv1.slots)))
                zrep = load_zrep(z_v1, vt)
                re_, ro_ = finalize(ae, ao, (0, vt // 2), (0, vt // 2),
                                    ("cvi_e", "cvi_o"), zrep)
                ov = out_xv2.ap().rearrange("(t p) d -> p t d", p=P)
                ge = vt // 2
                for buf, par in ((re_, 0), (ro_, 1)):
                    for g0 in range(0, ge, 8):
                        gn = min(8, ge - g0)
                        of = workp.tile([P, 8, EMB], F32, tag="out_f32")
                        nc.vector.tensor_copy(out=of[:, :gn, :], in_=buf[:, g0:g0 + gn, :])
                        nc.sync.dma_start(
                            out=ov[:, 2 * g0 + par:min(2 * (g0 + gn) + par, vt):2, :],
                            in_=of[:, :gn, :])

        nc.compile()
        return nc

    def assemble(self, results):
        out = np.concatenate([results[k]["out_xv2"][: self.Vsh] for k in range(self.ncores)], 0)
        return out



# ---------------------------------------------------------------- entry points

_CACHE = {}


def _get_built(edge_index):
    key = hash(np.asarray(edge_index).tobytes())
    if key not in _CACHE:
        pr = Problem(100000, 200000, 5, 19)
        pr.prep(np.asarray(edge_index))
        _CACHE.clear()
        _CACHE[key] = (pr, pr.build())
    return _CACHE[key]


def kernel(**inputs):
    pr, nc = _get_built(inputs["edge_index"])
    in_maps = pr.in_maps(inputs)
    from concourse.bass_utils import run_bass_kernel_spmd
    res = run_bass_kernel_spmd(nc, in_maps, core_ids=list(range(pr.ncores)))
    return pr.assemble(res.results).astype(np.float32)


def _pjrt_fn(nc, n_cores, nchain=1):
    """Mirror bass2jax.run_bass_via_pjrt but return a reusable jitted fn
    (no donation) plus the input-name layout, for steady-state timing."""
    import jax
    import concourse.mybir as mb
    from concourse import bass2jax
    from concourse.bass2jax import _bass_exec_p, partition_id_tensor, install_neuronx_cc_hook
    from jax.sharding import Mesh, PartitionSpec
    from jax.experimental.shard_map import shard_map
    install_neuronx_cc_hook()
    partition_name = nc.partition_id_tensor.name if nc.partition_id_tensor else None
    in_names, out_names, out_avals, zero_outs = [], [], [], []
    for alloc in nc.m.functions[0].allocations:
        if not isinstance(alloc, mb.MemoryLocationSet):
            continue
        name = alloc.memorylocations[0].name
        if alloc.kind == "ExternalInput":
            if name != partition_name:
                in_names.append(name)
        elif alloc.kind == "ExternalOutput":
            out_names.append(name)
            shape = tuple(alloc.tensor_shape)
            dtype = mb.dt.np(alloc.dtype)
            out_avals.append(jax.core.ShapedArray(shape, dtype))
            zero_outs.append(np.zeros(shape, dtype))
    n_params = len(in_names)
    all_names = in_names + out_names
    if partition_name is not None:
        all_names_full = all_names + [partition_name]
    def _body(*args):
        params = list(args[:n_params])
        outs = tuple(args[n_params:])
        for _ in range(nchain):
            operands = params + list(outs)
            if partition_name is not None:
                operands.append(partition_id_tensor())
            outs = _bass_exec_p.bind(
                *operands, out_avals=tuple(out_avals),
                in_names=tuple(all_names if partition_name is None else all_names + [partition_name]),
                out_names=tuple(out_names), lowering_input_output_aliases=(),
                sim_require_finite=False, sim_require_nnan=False, nc=nc)
        return tuple(outs)
    devices = jax.devices()[:n_cores]
    mesh = Mesh(np.asarray(devices), ("core",))
    in_specs = (PartitionSpec("core"),) * (n_params + len(out_names))
    out_specs = (PartitionSpec("core"),) * len(out_names)
    fn = jax.jit(shard_map(_body, mesh=mesh, in_specs=in_specs, out_specs=out_specs,
                           check_rep=False), keep_unused=True)
    return fn, in_names, out_names, zero_outs


def run_timed(inputs, iters=4, nchain=6):
    """Returns (full_output, dict with per-exec estimate)."""
    import jax, time
    pr, nc = _get_built(inputs["edge_index"])
    in_maps = pr.in_maps(inputs)
    fn1, in_names, out_names, zero_outs = _pjrt_fn(nc, pr.ncores, nchain=1)
    concat_in = [np.concatenate([np.asarray(in_maps[c][n]) for c in range(pr.ncores)], 0)
                 for n in in_names]
    concat_zero = [np.zeros((pr.ncores * z.shape[0],) + z.shape[1:], z.dtype) for z in zero_outs]
    dev_args = [jax.device_put(a) for a in concat_in + concat_zero]
    out = fn1(*dev_args)
    jax.block_until_ready(out)
    t1s = []
    for _ in range(iters):
        t0 = time.perf_counter()
        out = fn1(*dev_args)
        jax.block_until_ready(out)
        t1s.append(time.perf_counter() - t0)
    times = {"t1": t1s, "tN": t1s, "nchain": 1, "per_exec_s": min(t1s)}
    arrs = [np.asarray(o) for o in out]
    results = []
    for c in range(pr.ncores):
        d = {}
        for i, n in enumerate(out_names):
            per = arrs[i].reshape(pr.ncores, arrs[i].shape[0] // pr.ncores, *arrs[i].shape[1:])
            d[n] = per[c]
        results.append(d)
    return pr.assemble(results).astype(np.float32), times


def predicted_ns(inputs):
    """Cost-model estimate via no-exec CoreSim (core 0)."""
    from concourse.bass_interp import CoreSim
    pr, nc = _get_built(inputs["edge_index"])
    sim = CoreSim(nc, no_exec=True)
    sim.event_loop()
    return sim.time
